# revision 3
# baseline (speedup 1.0000x reference)
"""Additive attention (B=4, C=256, CO=64, H=W=24) on 8 TRN2 NeuronCores.

Sharding: core i handles batch b = i // 2 and Nq-half h = i % 2 (rows
12h..12h+12 of the 24x24 query grid). Each core produces a complete
(256, 288) slice of the output; no collectives are needed.

Per-core math (Nk=576, Nq=288, CO=64):
  k_ = Wk @ key_b   (64, 576);  q_ = Wq @ qry_bh  (64, 288)
  scores[k, q] = sum_c wf[c] * tanh(k_[c, k] + q_[c, q] + bk[c] + bq[c]) + bf
  attn = sigmoid(scores);  out = value_b @ attn^T-ish -> (256, 288)

Layout trick: partitions = 128 = two stacked copies of the 64 channels, so
one ACT instruction (bias = per-partition scalar) evaluates tanh for TWO
q's over all 576 k; a (128, 2) wf2 matmul then reduces channels for both
q's at once, writing scores straight into (k, q) PSUM tiles.
"""

import numpy as np

B, C, CO, HW, NK = 4, 256, 64, 24, 576
NQ = 288  # per-core query count (half of 576)
NPAIR = NQ // 2  # 144 q-pairs
KT_SIZES = [128, 128, 128, 128, 64]  # 576 split into partition tiles

_cache = {}


def _build(mode="bias", chunk=16):
    import concourse.bacc as bacc
    import concourse.mybir as mybir
    from concourse.tile import TileContext
    from concourse.masks import make_identity

    f32 = mybir.dt.float32
    bf16 = mybir.dt.bfloat16
    AF = mybir.ActivationFunctionType

    nc = bacc.Bacc("TRN2", target_bir_lowering=False, debug=False, num_devices=8)
    keyb = nc.dram_tensor("keyb", [C, NK], f32, kind="ExternalInput")
    qryb = nc.dram_tensor("qryb", [C, NQ], f32, kind="ExternalInput")
    valb = nc.dram_tensor("valb", [C, NK], f32, kind="ExternalInput")
    wkt2 = nc.dram_tensor("wkt2", [C, 128], f32, kind="ExternalInput")
    wqt = nc.dram_tensor("wqt", [C, CO], f32, kind="ExternalInput")
    bqk2 = nc.dram_tensor("bqk2", [128, 1], f32, kind="ExternalInput")
    wf2 = nc.dram_tensor("wf2", [128, 2], bf16, kind="ExternalInput")
    bf2 = nc.dram_tensor("bf2", [128, 1], f32, kind="ExternalInput")
    out = nc.dram_tensor("out", [C, NQ], f32, kind="ExternalOutput")

    with TileContext(nc) as tc:
        with (
            tc.tile_pool(name="consts", bufs=1) as consts,
            tc.tile_pool(name="inp", bufs=1) as inp,
            tc.tile_pool(name="work", bufs=1) as work,
            tc.tile_pool(name="psc", bufs=1, space="PSUM") as psc,
            tc.tile_pool(name="ppro", bufs=1, space="PSUM") as ppro,
            tc.tile_pool(name="pout", bufs=2, space="PSUM") as pout,
        ):
            # ---- DMA inputs ----
            key_sb = [inp.tile([128, NK], f32, tag=f"key{t}", name=f"key{t}") for t in range(2)]
            qry_sb = [inp.tile([128, NQ], f32, tag=f"qry{t}", name=f"qry{t}") for t in range(2)]
            val_sb = [inp.tile([128, NK], f32, tag=f"val{t}", name=f"val{t}") for t in range(2)]
            wkt2_sb = [consts.tile([128, 128], f32, tag=f"wkt{t}", name=f"wkt{t}") for t in range(2)]
            wqt_sb = [consts.tile([128, CO], f32, tag=f"wqt{t}", name=f"wqt{t}") for t in range(2)]
            bqk2_sb = consts.tile([128, 1], f32, tag="bqk2")
            wf2_sb = consts.tile([128, 2], bf16, tag="wf2")
            bf2_sb = consts.tile([128, 1], f32, tag="bf2")
            ident = consts.tile([128, 128], f32, tag="ident")
            for t in range(2):
                sl = slice(t * 128, (t + 1) * 128)
                nc.sync.dma_start(out=key_sb[t][:], in_=keyb.ap()[sl, :])
                nc.sync.dma_start(out=qry_sb[t][:], in_=qryb.ap()[sl, :])
                nc.sync.dma_start(out=val_sb[t][:], in_=valb.ap()[sl, :])
                nc.sync.dma_start(out=wkt2_sb[t][:], in_=wkt2.ap()[sl, :])
                nc.sync.dma_start(out=wqt_sb[t][:], in_=wqt.ap()[sl, :])
            nc.sync.dma_start(out=bqk2_sb[:], in_=bqk2.ap())
            nc.sync.dma_start(out=wf2_sb[:], in_=wf2.ap())
            nc.sync.dma_start(out=bf2_sb[:], in_=bf2.ap())
            make_identity(nc, ident[:])

            # ---- k2 = [WkT; WkT]^T @ key  -> (128, 576) bf16 ----
            k2_sb = work.tile([128, NK], bf16, tag="k2")
            for half in range(2):
                pk2 = ppro.tile([128, NQ], f32, tag="ppro")
                csl = slice(half * NQ, (half + 1) * NQ)
                for ct in range(2):
                    nc.tensor.matmul(
                        out=pk2[:],
                        lhsT=wkt2_sb[ct][:],
                        rhs=key_sb[ct][:, csl],
                        start=(ct == 0),
                        stop=(ct == 1),
                    )
                nc.vector.tensor_copy(out=k2_sb[:, csl], in_=pk2[:])

            # ---- qbias (128, 144): col j = [q_(2j); q_(2j+1)] + bq + bk ----
            pqb = ppro.tile([128, NPAIR], f32, tag="ppro")
            for par in range(2):  # even / odd q columns
                for ct in range(2):
                    nc.tensor.matmul(
                        out=pqb[par * CO : (par + 1) * CO, :],
                        lhsT=wqt_sb[ct][:],
                        rhs=qry_sb[ct][:, par : NQ : 2],
                        start=(ct == 0),
                        stop=(ct == 1),
                    )
            qbias = work.tile([128, NPAIR], f32, tag="qbias")
            nc.vector.tensor_scalar_add(out=qbias[:], in0=pqb[:], scalar1=bqk2_sb[:])

            # ---- value transpose -> vT (k, cv) bf16 tiles ----
            vt_sb = [
                work.tile([KT_SIZES[kt], 2 * 128], bf16, tag=f"vt{kt}", name=f"vt{kt}")
                for kt in range(5)
            ]
            for kt in range(5):
                pvt = ppro.tile([KT_SIZES[kt], 2 * 128], f32, tag="ppro")
                ks = slice(kt * 128, kt * 128 + KT_SIZES[kt])
                for ct in range(2):
                    nc.tensor.transpose(
                        out=pvt[:, ct * 128 : (ct + 1) * 128],
                        in_=val_sb[ct][:, ks],
                        identity=ident[:],
                    )
                nc.vector.tensor_copy(out=vt_sb[kt][:], in_=pvt[:])

            # ---- main loop: tanh + channel-reduce matmul ----
            psc_t = [psc.tile([KT_SIZES[kt], NQ], f32, tag=f"sc{kt}", name=f"sc{kt}") for kt in range(5)]
            if mode == "bias":
                with tc.tile_pool(name="pre", bufs=3) as prep:
                    for j in range(NPAIR):
                        pre2 = prep.tile([128, NK], bf16, tag="pre2")
                        nc.scalar.activation(
                            pre2[:], k2_sb[:], AF.Tanh, bias=qbias[:, j : j + 1]
                        )
                        for kt in range(5):
                            ks = slice(kt * 128, kt * 128 + KT_SIZES[kt])
                            nc.tensor.matmul(
                                out=psc_t[kt][:, 2 * j : 2 * j + 2],
                                lhsT=pre2[:, ks],
                                rhs=wf2_sb[:],
                                start=True,
                                stop=True,
                            )
            else:  # dve: vector engine does the adds, ACT runs big tanh instrs
                nchunk = (NPAIR + chunk - 1) // chunk
                with tc.tile_pool(name="pre", bufs=2) as prep:
                    for cidx in range(nchunk):
                        j0 = cidx * chunk
                        j1 = min(j0 + chunk, NPAIR)
                        pre = prep.tile([128, chunk * NK], bf16, tag="pre")
                        for j in range(j0, j1):
                            sl = slice((j - j0) * NK, (j - j0 + 1) * NK)
                            nc.vector.tensor_scalar_add(
                                out=pre[:, sl], in0=k2_sb[:], scalar1=qbias[:, j : j + 1]
                            )
                        nc.scalar.activation(
                            pre[:, : (j1 - j0) * NK],
                            pre[:, : (j1 - j0) * NK],
                            AF.Tanh,
                        )
                        for j in range(j0, j1):
                            for kt in range(5):
                                ks = slice(
                                    (j - j0) * NK + kt * 128,
                                    (j - j0) * NK + kt * 128 + KT_SIZES[kt],
                                )
                                nc.tensor.matmul(
                                    out=psc_t[kt][:, 2 * j : 2 * j + 2],
                                    lhsT=pre[:, ks],
                                    rhs=wf2_sb[:],
                                    start=True,
                                    stop=True,
                                )

            # ---- sigmoid -> attn bf16 (k, q) ----
            attn_sb = [
                work.tile([KT_SIZES[kt], NQ], bf16, tag=f"attn{kt}", name=f"attn{kt}") for kt in range(5)
            ]
            for kt in range(5):
                nc.scalar.activation(
                    attn_sb[kt][:],
                    psc_t[kt][:],
                    AF.Sigmoid,
                    bias=bf2_sb[: KT_SIZES[kt], :],
                )

            # ---- out = value @ attn : (256, 288) ----
            for ct in range(2):
                po = pout.tile([128, NQ], f32, tag="pout")
                for kt in range(5):
                    nc.tensor.matmul(
                        out=po[:],
                        lhsT=vt_sb[kt][:, ct * 128 : (ct + 1) * 128],
                        rhs=attn_sb[kt][:],
                        start=(kt == 0),
                        stop=(kt == 4),
                    )
                o_sb = work.tile([128, NQ], f32, tag=f"osb{ct}", name=f"osb{ct}")
                nc.vector.tensor_copy(out=o_sb[:], in_=po[:])
                nc.sync.dma_start(
                    out=out.ap()[ct * 128 : (ct + 1) * 128, :], in_=o_sb[:]
                )

    nc.finalize()
    return nc


def _prep_in_maps(key, query, value, Wk, bk, Wq, bq, wf, bf):
    import ml_dtypes

    f32 = np.float32
    key = np.ascontiguousarray(key, f32).reshape(B, C, NK)
    query = np.ascontiguousarray(query, f32).reshape(B, C, HW, HW)
    value = np.ascontiguousarray(value, f32).reshape(B, C, NK)
    WkT2 = np.ascontiguousarray(
        np.concatenate([np.asarray(Wk, f32).T] * 2, axis=1)
    )  # (256, 128)
    WqT = np.ascontiguousarray(np.asarray(Wq, f32).T)  # (256, 64)
    bqk2 = np.tile(np.asarray(bk, f32) + np.asarray(bq, f32), 2).reshape(128, 1)
    wf2 = np.zeros((128, 2), f32)
    wf2[:CO, 0] = np.asarray(wf, f32)
    wf2[CO:, 1] = np.asarray(wf, f32)
    wf2 = wf2.astype(ml_dtypes.bfloat16)
    bf2 = np.full((128, 1), np.float32(bf), f32)

    in_maps = []
    for i in range(8):
        b, h = i // 2, i % 2
        qs = np.ascontiguousarray(query[b, :, h * 12 : (h + 1) * 12, :]).reshape(C, NQ)
        in_maps.append(
            {
                "keyb": key[b],
                "qryb": qs,
                "valb": value[b],
                "wkt2": WkT2,
                "wqt": WqT,
                "bqk2": np.ascontiguousarray(bqk2),
                "wf2": wf2,
                "bf2": bf2,
            }
        )
    return in_maps


def run(mode="bias", chunk=16, trace=False, **inputs):
    from concourse.bass_utils import run_bass_kernel_spmd

    cache_key = (mode, chunk)
    if cache_key not in _cache:
        _cache[cache_key] = _build(mode, chunk)
    nc = _cache[cache_key]
    in_maps = _prep_in_maps(**inputs)
    res = run_bass_kernel_spmd(
        nc, in_maps, core_ids=list(range(8)), trace=trace
    )
    out = np.empty((B, C, HW, HW), np.float32)
    for i in range(8):
        b, h = i // 2, i % 2
        out[b, :, h * 12 : (h + 1) * 12, :] = res.results[i]["out"].reshape(C, 12, HW)
    return out, res


def kernel(**inputs):
    out, _ = run(mode="bias", **inputs)
    return out


# revision 14
# speedup vs baseline: 1.0104x; 1.0104x over previous
"""Additive attention (B=4, C=256, CO=64, H=W=24) on 8 TRN2 NeuronCores.

Sharding: core i handles batch b = i // 2 and Nq-half h = i % 2 (rows
12h..12h+12 of the 24x24 query grid). Each core produces a complete
(256, 288) slice of the output; no collectives are needed.

Per-core math (Nk=576, Nq=288, CO=64):
  k_ = Wk @ key_b   (64, 576);  q_ = Wq @ qry_bh  (64, 288)
  scores[k, q] = sum_c wf[c] * tanh(k_[c, k] + q_[c, q] + bk[c] + bq[c]) + bf
  attn = sigmoid(scores);  out = value_b @ attn -> (256, 288)

"rep" layout (the fast path): partitions = 4 channel-rows x 32 q's
(c-major): partition p = 32*rho + u holds channel c = 4*s + rho for query
q = 32*G + u at channel-step s. The k_ rows are replicated 32x across
partitions via a DRAM round-trip DMA, the q_ column enters as the
per-partition scalar of a vector-engine add, tanh runs in big ACT
instructions, and a block-diagonal wf stationary reduces channels with
16 accumulating matmuls per query group -- scores land compact
(32 q, 576 k) in PSUM with full partition utilization everywhere.
"""

import numpy as np

B, C, CO, HW, NK = 4, 256, 64, 24, 576
NQ = 288  # per-core query count (half of 576)
NPAIR = NQ // 2
KT_SIZES = [128, 128, 128, 128, 64]  # 576 split into partition tiles
RP = 4  # channel rows per partition block ("rep" mode)
NG = 32  # q's per group
NS = CO // RP  # 16 channel steps
NGRP = NQ // NG  # 9 q groups

_cache = {}


def _build_rep(nc, mybir, tc, consts, inp, work):
    from concourse.masks import make_identity

    f32 = mybir.dt.float32
    bf16 = mybir.dt.bfloat16
    AF = mybir.ActivationFunctionType

    keyb = nc.dram_tensor("keyb", [C, NK], f32, kind="ExternalInput")
    qryb = nc.dram_tensor("qryb", [C, NQ], f32, kind="ExternalInput")
    valb = nc.dram_tensor("valb", [C, NK], f32, kind="ExternalInput")
    wkt = nc.dram_tensor("wkt", [C, CO], f32, kind="ExternalInput")
    wqt = nc.dram_tensor("wqt", [C, CO], f32, kind="ExternalInput")
    bqk = nc.dram_tensor("bqk", [CO, 1], f32, kind="ExternalInput")
    wf32p = nc.dram_tensor("wf32p", [128, NS * NG], bf16, kind="ExternalInput")
    bf2 = nc.dram_tensor("bf2", [128, 1], f32, kind="ExternalInput")
    out = nc.dram_tensor("out", [C, NQ], f32, kind="ExternalOutput")
    k2scr = nc.dram_tensor("k2scr", [CO, NK], bf16, kind="Internal")

    # ---- DMA inputs ----
    key_sb = [inp.tile([128, NK], f32, tag=f"key{t}", name=f"key{t}") for t in range(2)]
    qry_sb = [inp.tile([128, NQ], f32, tag=f"qry{t}", name=f"qry{t}") for t in range(2)]
    val_sb = [inp.tile([128, NK], f32, tag=f"val{t}", name=f"val{t}") for t in range(2)]
    wkt_sb = [inp.tile([128, CO], f32, tag=f"wkt{t}", name=f"wkt{t}") for t in range(2)]
    wqt_sb = [inp.tile([128, CO], f32, tag=f"wqt{t}", name=f"wqt{t}") for t in range(2)]
    bqk_sb = consts.tile([CO, 1], f32, tag="bqk")
    wf32p_sb = consts.tile([128, NS * NG], bf16, tag="wf32p")
    bf2_sb = consts.tile([128, 1], f32, tag="bf2")
    ident = consts.tile([128, 128], f32, tag="ident")
    ident_bf = consts.tile([NG, NG], bf16, tag="ident_bf")
    for t in range(2):
        sl = slice(t * 128, (t + 1) * 128)
        nc.sync.dma_start(out=key_sb[t][:], in_=keyb.ap()[sl, :])
        nc.sync.dma_start(out=qry_sb[t][:], in_=qryb.ap()[sl, :])
        nc.sync.dma_start(out=val_sb[t][:], in_=valb.ap()[sl, :])
        nc.sync.dma_start(out=wkt_sb[t][:], in_=wkt.ap()[sl, :])
        nc.sync.dma_start(out=wqt_sb[t][:], in_=wqt.ap()[sl, :])
    nc.sync.dma_start(out=bqk_sb[:], in_=bqk.ap())
    nc.sync.dma_start(out=wf32p_sb[:], in_=wf32p.ap())
    nc.sync.dma_start(out=bf2_sb[:], in_=bf2.ap())
    make_identity(nc, ident[:])
    make_identity(nc, ident_bf[:])

    ksbig = work.tile([128, NS * NK], bf16, tag="ksbig")
    qb_big = work.tile([128, NGRP * NS], f32, tag="qb_big")
    k2n_sb = work.tile([CO, NK], bf16, tag="k2n")
    qn_sb = work.tile([CO, NQ], f32, tag="qn")
    vt_sb = [
        work.tile([KT_SIZES[kt], 2 * 128], bf16, tag=f"vt{kt}", name=f"vt{kt}")
        for kt in range(5)
    ]
    attn_sb = [
        work.tile([KT_SIZES[kt], NQ], bf16, tag=f"attn{kt}", name=f"attn{kt}")
        for kt in range(5)
    ]

    with tc.tile_pool(name="ppro", bufs=2, space="PSUM") as ppro:
        # ---- k_ = WkT^T @ key -> (64, 576) bf16, then to DRAM scratch ----
        for half in range(2):
            pk2 = ppro.tile([CO, NQ], f32, tag="ppro")
            csl = slice(half * NQ, (half + 1) * NQ)
            for ct in range(2):
                nc.tensor.matmul(
                    out=pk2[:],
                    lhsT=wkt_sb[ct][:],
                    rhs=key_sb[ct][:, csl],
                    start=(ct == 0),
                    stop=(ct == 1),
                )
            nc.vector.tensor_copy(out=k2n_sb[:, csl], in_=pk2[:])
        nc.sync.dma_start(out=k2scr.ap(), in_=k2n_sb[:])

        # ---- q_ + bq + bk -> (64, 288) f32 ----
        pqn = ppro.tile([CO, NQ], f32, tag="ppro")
        for ct in range(2):
            nc.tensor.matmul(
                out=pqn[:],
                lhsT=wqt_sb[ct][:],
                rhs=qry_sb[ct][:],
                start=(ct == 0),
                stop=(ct == 1),
            )
        nc.vector.tensor_scalar_add(out=qn_sb[:], in0=pqn[:], scalar1=bqk_sb[:])

        # ---- replicate k rows: ksbig[32*rho + u, s*576 + k] = k_[16*rho + s, k]
        # (channel of (band rho, step s) is c = 16*rho + s, block-contiguous,
        # so each band's source is a plain row range of k2scr; the gpsimd
        # queue issues these so the SP queue stays free for qb scatter) ----
        for rho in range(RP):
            for sh in range(2):
                nsh = NS // 2
                dst = ksbig[
                    NG * rho : NG * (rho + 1), sh * nsh * NK : (sh + 1) * nsh * NK
                ].rearrange("p (s k) -> p s k", k=NK)
                src = (
                    k2scr.ap()[NS * rho + sh * nsh : NS * rho + (sh + 1) * nsh, :]
                    .rearrange("s k -> () s k")
                    .broadcast_to((NG, nsh, NK))
                )
                nc.gpsimd.dma_start(out=dst, in_=src)

        # ---- qRT = q_^T (q-part, c-free) via PE transpose ----
        qrt = work.tile([128, 3 * CO], f32, tag="qrt")
        for t in range(3):
            qsz = 128 if t < 2 else 32
            pqt = ppro.tile([128, CO], f32, tag="ppro")
            nc.tensor.transpose(
                out=pqt[:qsz, :],
                in_=qn_sb[:, t * 128 : t * 128 + qsz],
                identity=ident[:CO, :CO],
            )
            nc.vector.tensor_copy(
                out=qrt[:qsz, t * CO : (t + 1) * CO], in_=pqt[:qsz, :]
            )

        # ---- qb_big[32*rho + u, 16*G + s] = q_[16*rho + s, 32*G + u]
        # = qrt[32*(G%4) + u, 64*(G//4) + 16*rho + s]; 16 SBUF->SBUF DMAs ----
        for rho in range(RP):
            for w in range(4):
                nt = 3 if w == 0 else 2  # G = 4t + w must stay < 9
                dst = qb_big[
                    NG * rho : NG * (rho + 1), 16 * w : 16 * w + 64 * (nt - 1) + 16
                ].rearrange("p (t s) -> p t s", s=NS)[:, :: RP, :]
                src = qrt[
                    NG * w : NG * (w + 1), 16 * rho : 16 * rho + 64 * (nt - 1) + 16
                ].rearrange("p (t s) -> p t s", s=NS)[:, :: RP, :]
                nc.sync.dma_start(out=dst, in_=src)

        # ---- value transpose -> vT (k, cv) bf16 tiles ----
        for kt in range(5):
            pvt = ppro.tile([KT_SIZES[kt], 2 * 128], f32, tag="ppro")
            ks = slice(kt * 128, kt * 128 + KT_SIZES[kt])
            for ct in range(2):
                nc.tensor.transpose(
                    out=pvt[:, ct * 128 : (ct + 1) * 128],
                    in_=val_sb[ct][:, ks],
                    identity=ident[:],
                )
            nc.vector.tensor_copy(out=vt_sb[kt][:], in_=pvt[:])

    # ---- main loop over 9 q-groups ----
    with (
        tc.tile_pool(name="pre", bufs=2) as prep,
        tc.tile_pool(name="aq", bufs=2) as aqp,
        tc.tile_pool(name="scp", bufs=2, space="PSUM") as scp,
        tc.tile_pool(name="patt", bufs=2, space="PSUM") as pattp,
    ):
        for G in range(NGRP):
            pre = prep.tile([128, NS * NK], bf16, tag="pre")
            for s in range(NS):
                nc.vector.tensor_scalar_add(
                    out=pre[:, s * NK : (s + 1) * NK],
                    in0=ksbig[:, s * NK : (s + 1) * NK],
                    scalar1=qb_big[:, NS * G + s : NS * G + s + 1],
                )
            nc.scalar.activation(pre[:], pre[:], AF.Tanh)
            scg = scp.tile([NG, NK], f32, tag="scg")
            for c0, c1 in ((0, 512), (512, NK)):
                for s in range(NS):
                    nc.tensor.matmul(
                        out=scg[:, c0:c1],
                        lhsT=wf32p_sb[:, NG * s : NG * (s + 1)],
                        rhs=pre[:, s * NK + c0 : s * NK + c1],
                        start=(s == 0),
                        stop=(s == NS - 1),
                    )
            attn_q = aqp.tile([NG, NK], bf16, tag="attn_q")
            nc.scalar.activation(
                attn_q[:], scg[:], AF.Sigmoid, bias=bf2_sb[:NG, :]
            )
            for kt in range(5):
                ks = slice(kt * 128, kt * 128 + KT_SIZES[kt])
                patt = pattp.tile([KT_SIZES[kt], NG], bf16, tag="patt")
                nc.tensor.transpose(
                    out=patt[:], in_=attn_q[:, ks], identity=ident_bf[:]
                )
                nc.vector.tensor_copy(
                    out=attn_sb[kt][:, NG * G : NG * (G + 1)], in_=patt[:]
                )

    # ---- out = value @ attn : (256, 288) ----
    with tc.tile_pool(name="pout", bufs=2, space="PSUM") as pout:
        for ct in range(2):
            po = pout.tile([128, NQ], f32, tag="pout")
            for kt in range(5):
                nc.tensor.matmul(
                    out=po[:],
                    lhsT=vt_sb[kt][:, ct * 128 : (ct + 1) * 128],
                    rhs=attn_sb[kt][:],
                    start=(kt == 0),
                    stop=(kt == 4),
                )
            o_sb = work.tile([128, NQ], f32, tag=f"osb{ct}", name=f"osb{ct}")
            nc.vector.tensor_copy(out=o_sb[:], in_=po[:])
            nc.sync.dma_start(out=out.ap()[ct * 128 : (ct + 1) * 128, :], in_=o_sb[:])


def _build_pair(nc, mybir, tc, consts, inp, work, mode, chunk):
    """Older 2x64 pair layouts: mode 'bias' (ACT bias adds) or 'dve'."""
    from concourse.masks import make_identity

    f32 = mybir.dt.float32
    bf16 = mybir.dt.bfloat16
    AF = mybir.ActivationFunctionType

    keyb = nc.dram_tensor("keyb", [C, NK], f32, kind="ExternalInput")
    qryb = nc.dram_tensor("qryb", [C, NQ], f32, kind="ExternalInput")
    valb = nc.dram_tensor("valb", [C, NK], f32, kind="ExternalInput")
    wkt2 = nc.dram_tensor("wkt2", [C, 128], f32, kind="ExternalInput")
    wqt = nc.dram_tensor("wqt", [C, CO], f32, kind="ExternalInput")
    bqk2 = nc.dram_tensor("bqk2", [128, 1], f32, kind="ExternalInput")
    wf2 = nc.dram_tensor("wf2", [128, 2], bf16, kind="ExternalInput")
    bf2 = nc.dram_tensor("bf2", [128, 1], f32, kind="ExternalInput")
    out = nc.dram_tensor("out", [C, NQ], f32, kind="ExternalOutput")

    key_sb = [inp.tile([128, NK], f32, tag=f"key{t}", name=f"key{t}") for t in range(2)]
    qry_sb = [inp.tile([128, NQ], f32, tag=f"qry{t}", name=f"qry{t}") for t in range(2)]
    val_sb = [inp.tile([128, NK], f32, tag=f"val{t}", name=f"val{t}") for t in range(2)]
    wkt2_sb = [consts.tile([128, 128], f32, tag=f"wkt{t}", name=f"wkt{t}") for t in range(2)]
    wqt_sb = [consts.tile([128, CO], f32, tag=f"wqt{t}", name=f"wqt{t}") for t in range(2)]
    bqk2_sb = consts.tile([128, 1], f32, tag="bqk2")
    wf2_sb = consts.tile([128, 2], bf16, tag="wf2")
    bf2_sb = consts.tile([128, 1], f32, tag="bf2")
    ident = consts.tile([128, 128], f32, tag="ident")
    for t in range(2):
        sl = slice(t * 128, (t + 1) * 128)
        nc.sync.dma_start(out=key_sb[t][:], in_=keyb.ap()[sl, :])
        nc.sync.dma_start(out=qry_sb[t][:], in_=qryb.ap()[sl, :])
        nc.sync.dma_start(out=val_sb[t][:], in_=valb.ap()[sl, :])
        nc.sync.dma_start(out=wkt2_sb[t][:], in_=wkt2.ap()[sl, :])
        nc.sync.dma_start(out=wqt_sb[t][:], in_=wqt.ap()[sl, :])
    nc.sync.dma_start(out=bqk2_sb[:], in_=bqk2.ap())
    nc.sync.dma_start(out=wf2_sb[:], in_=wf2.ap())
    nc.sync.dma_start(out=bf2_sb[:], in_=bf2.ap())
    make_identity(nc, ident[:])

    with tc.tile_pool(name="ppro", bufs=1, space="PSUM") as ppro:
        k2_sb = work.tile([128, NK], bf16, tag="k2")
        for half in range(2):
            pk2 = ppro.tile([128, NQ], f32, tag="ppro")
            csl = slice(half * NQ, (half + 1) * NQ)
            for ct in range(2):
                nc.tensor.matmul(
                    out=pk2[:],
                    lhsT=wkt2_sb[ct][:],
                    rhs=key_sb[ct][:, csl],
                    start=(ct == 0),
                    stop=(ct == 1),
                )
            nc.vector.tensor_copy(out=k2_sb[:, csl], in_=pk2[:])

        pqb = ppro.tile([128, NPAIR], f32, tag="ppro")
        for par in range(2):
            for ct in range(2):
                nc.tensor.matmul(
                    out=pqb[par * CO : (par + 1) * CO, :],
                    lhsT=wqt_sb[ct][:],
                    rhs=qry_sb[ct][:, par : NQ : 2],
                    start=(ct == 0),
                    stop=(ct == 1),
                )
        qbias = work.tile([128, NPAIR], f32, tag="qbias")
        nc.vector.tensor_scalar_add(out=qbias[:], in0=pqb[:], scalar1=bqk2_sb[:])

        vt_sb = [
            work.tile([KT_SIZES[kt], 2 * 128], bf16, tag=f"vt{kt}", name=f"vt{kt}")
            for kt in range(5)
        ]
        for kt in range(5):
            pvt = ppro.tile([KT_SIZES[kt], 2 * 128], f32, tag="ppro")
            ks = slice(kt * 128, kt * 128 + KT_SIZES[kt])
            for ct in range(2):
                nc.tensor.transpose(
                    out=pvt[:, ct * 128 : (ct + 1) * 128],
                    in_=val_sb[ct][:, ks],
                    identity=ident[:],
                )
            nc.vector.tensor_copy(out=vt_sb[kt][:], in_=pvt[:])

    attn_sb = [
        work.tile([KT_SIZES[kt], NQ], bf16, tag=f"attn{kt}", name=f"attn{kt}")
        for kt in range(5)
    ]

    with tc.tile_pool(name="psc", bufs=1, space="PSUM") as psc:
        psc_t = [
            psc.tile([KT_SIZES[kt], NQ], f32, tag=f"sc{kt}", name=f"sc{kt}")
            for kt in range(5)
        ]
        if mode == "bias":
            with tc.tile_pool(name="pre", bufs=3) as prep:
                for j in range(NPAIR):
                    pre2 = prep.tile([128, NK], bf16, tag="pre2")
                    nc.scalar.activation(
                        pre2[:], k2_sb[:], AF.Tanh, bias=qbias[:, j : j + 1]
                    )
                    for kt in range(5):
                        ks = slice(kt * 128, kt * 128 + KT_SIZES[kt])
                        nc.tensor.matmul(
                            out=psc_t[kt][:, 2 * j : 2 * j + 2],
                            lhsT=pre2[:, ks],
                            rhs=wf2_sb[:],
                            start=True,
                            stop=True,
                        )
        else:  # dve
            nchunk = (NPAIR + chunk - 1) // chunk
            with tc.tile_pool(name="pre", bufs=2) as prep:
                for cidx in range(nchunk):
                    j0 = cidx * chunk
                    j1 = min(j0 + chunk, NPAIR)
                    pre = prep.tile([128, chunk * NK], bf16, tag="pre")
                    for j in range(j0, j1):
                        sl = slice((j - j0) * NK, (j - j0 + 1) * NK)
                        nc.vector.tensor_scalar_add(
                            out=pre[:, sl], in0=k2_sb[:], scalar1=qbias[:, j : j + 1]
                        )
                    nc.scalar.activation(
                        pre[:, : (j1 - j0) * NK], pre[:, : (j1 - j0) * NK], AF.Tanh
                    )
                    for j in range(j0, j1):
                        for kt in range(5):
                            ks = slice(
                                (j - j0) * NK + kt * 128,
                                (j - j0) * NK + kt * 128 + KT_SIZES[kt],
                            )
                            nc.tensor.matmul(
                                out=psc_t[kt][:, 2 * j : 2 * j + 2],
                                lhsT=pre[:, ks],
                                rhs=wf2_sb[:],
                                start=True,
                                stop=True,
                            )
        for kt in range(5):
            nc.scalar.activation(
                attn_sb[kt][:],
                psc_t[kt][:],
                AF.Sigmoid,
                bias=bf2_sb[: KT_SIZES[kt], :],
            )

    with tc.tile_pool(name="pout", bufs=2, space="PSUM") as pout:
        for ct in range(2):
            po = pout.tile([128, NQ], f32, tag="pout")
            for kt in range(5):
                nc.tensor.matmul(
                    out=po[:],
                    lhsT=vt_sb[kt][:, ct * 128 : (ct + 1) * 128],
                    rhs=attn_sb[kt][:],
                    start=(kt == 0),
                    stop=(kt == 4),
                )
            o_sb = work.tile([128, NQ], f32, tag=f"osb{ct}", name=f"osb{ct}")
            nc.vector.tensor_copy(out=o_sb[:], in_=po[:])
            nc.sync.dma_start(out=out.ap()[ct * 128 : (ct + 1) * 128, :], in_=o_sb[:])


def _build(mode="rep", chunk=12):
    import concourse.bacc as bacc
    import concourse.mybir as mybir
    from concourse.tile import TileContext

    nc = bacc.Bacc("TRN2", target_bir_lowering=False, debug=False, num_devices=8)
    with TileContext(nc) as tc:
        with (
            tc.tile_pool(name="consts", bufs=1) as consts,
            tc.tile_pool(name="inp", bufs=1) as inp,
            tc.tile_pool(name="work", bufs=1) as work,
        ):
            if mode == "rep":
                _build_rep(nc, mybir, tc, consts, inp, work)
            else:
                _build_pair(nc, mybir, tc, consts, inp, work, mode, chunk)
    nc.finalize()
    return nc


def _prep_in_maps(mode, key, query, value, Wk, bk, Wq, bq, wf, bf):
    import ml_dtypes

    f32 = np.float32
    key = np.ascontiguousarray(key, f32).reshape(B, C, NK)
    query = np.ascontiguousarray(query, f32).reshape(B, C, HW, HW)
    value = np.ascontiguousarray(value, f32).reshape(B, C, NK)
    WqT = np.ascontiguousarray(np.asarray(Wq, f32).T)  # (256, 64)
    bf2 = np.full((128, 1), np.float32(bf), f32)
    wf = np.asarray(wf, f32)

    common = {"wqt": WqT, "bf2": bf2}
    if mode == "rep":
        common["wkt"] = np.ascontiguousarray(np.asarray(Wk, f32).T)
        common["bqk"] = (np.asarray(bk, f32) + np.asarray(bq, f32)).reshape(CO, 1)
        wf32p = np.zeros((128, NS, NG), f32)
        for rho in range(RP):
            for s in range(NS):
                # channel of (band rho, step s) is 16*rho + s (block-contiguous)
                wf32p[NG * rho : NG * (rho + 1), s, :] = np.eye(NG, dtype=f32) * wf[
                    NS * rho + s
                ]
        common["wf32p"] = np.ascontiguousarray(
            wf32p.reshape(128, NS * NG).astype(ml_dtypes.bfloat16)
        )
    else:
        common["wkt2"] = np.ascontiguousarray(
            np.concatenate([np.asarray(Wk, f32).T] * 2, axis=1)
        )
        common["bqk2"] = np.ascontiguousarray(
            np.tile(np.asarray(bk, f32) + np.asarray(bq, f32), 2).reshape(128, 1)
        )
        wf2 = np.zeros((128, 2), f32)
        wf2[:CO, 0] = wf
        wf2[CO:, 1] = wf
        common["wf2"] = wf2.astype(ml_dtypes.bfloat16)

    in_maps = []
    for i in range(8):
        b, h = i // 2, i % 2
        qs = np.ascontiguousarray(query[b, :, h * 12 : (h + 1) * 12, :]).reshape(C, NQ)
        m = {"keyb": key[b], "qryb": qs, "valb": value[b]}
        m.update(common)
        in_maps.append(m)
    return in_maps


def run(mode="rep", chunk=12, trace=False, **inputs):
    from concourse.bass_utils import run_bass_kernel_spmd

    cache_key = (mode, chunk)
    if cache_key not in _cache:
        _cache[cache_key] = _build(mode, chunk)
    nc = _cache[cache_key]
    in_maps = _prep_in_maps(mode, **inputs)
    res = run_bass_kernel_spmd(nc, in_maps, core_ids=list(range(8)), trace=trace)
    out = np.empty((B, C, HW, HW), np.float32)
    for i in range(8):
        b, h = i // 2, i % 2
        out[b, :, h * 12 : (h + 1) * 12, :] = res.results[i]["out"].reshape(C, 12, HW)
    return out, res


def kernel(**inputs):
    out, _ = run(mode="rep", **inputs)
    return out


# revision 16
# speedup vs baseline: 1.1309x; 1.1193x over previous
"""Additive attention (B=4, C=256, CO=64, H=W=24) on 8 TRN2 NeuronCores.

Sharding: core i handles batch b = i // 2 and Nq-half h = i % 2 (rows
12h..12h+12 of the 24x24 query grid). Each core produces a complete
(256, 288) slice of the output; no collectives are needed.

Per-core math (Nk=576, Nq=288, CO=64):
  k_ = Wk @ key_b   (64, 576);  q_ = Wq @ qry_bh  (64, 288)
  scores[k, q] = sum_c wf[c] * tanh(k_[c, k] + q_[c, q] + bk[c] + bq[c]) + bf
  attn = sigmoid(scores);  out = value_b @ attn -> (256, 288)

"rep" layout (the fast path): partitions = 4 channel-rows x 32 q's
(c-major): partition p = 32*rho + u holds channel c = 4*s + rho for query
q = 32*G + u at channel-step s. The k_ rows are replicated 32x across
partitions via a DRAM round-trip DMA, the q_ column enters as the
per-partition scalar of a vector-engine add, tanh runs in big ACT
instructions, and a block-diagonal wf stationary reduces channels with
16 accumulating matmuls per query group -- scores land compact
(32 q, 576 k) in PSUM with full partition utilization everywhere.
"""

import numpy as np

B, C, CO, HW, NK = 4, 256, 64, 24, 576
NQ = 288  # per-core query count (half of 576)
NPAIR = NQ // 2
KT_SIZES = [128, 128, 128, 128, 64]  # 576 split into partition tiles
RP = 4  # channel rows per partition block ("rep" mode)
NG = 32  # q's per group
NS = CO // RP  # 16 channel steps
NGRP = NQ // NG  # 9 q groups

_cache = {}


def _build_rep(nc, mybir, tc, consts, inp, work):
    from concourse.masks import make_identity

    f32 = mybir.dt.float32
    bf16 = mybir.dt.bfloat16
    AF = mybir.ActivationFunctionType

    keyb = nc.dram_tensor("keyb", [C, NK], f32, kind="ExternalInput")
    qryb = nc.dram_tensor("qryb", [C, NQ], f32, kind="ExternalInput")
    valb = nc.dram_tensor("valb", [C, NK], f32, kind="ExternalInput")
    wkt = nc.dram_tensor("wkt", [C, CO], f32, kind="ExternalInput")
    wqt = nc.dram_tensor("wqt", [C, CO], f32, kind="ExternalInput")
    bqk = nc.dram_tensor("bqk", [CO, 1], f32, kind="ExternalInput")
    wf32p = nc.dram_tensor("wf32p", [128, NS * NG], bf16, kind="ExternalInput")
    bf2 = nc.dram_tensor("bf2", [128, 1], f32, kind="ExternalInput")
    out = nc.dram_tensor("out", [C, NQ], f32, kind="ExternalOutput")
    k2scr = nc.dram_tensor("k2scr", [CO, NK], bf16, kind="Internal")

    # ---- DMA inputs ----
    key_sb = [inp.tile([128, NK], f32, tag=f"key{t}", name=f"key{t}") for t in range(2)]
    qry_sb = [inp.tile([128, NQ], f32, tag=f"qry{t}", name=f"qry{t}") for t in range(2)]
    val_sb = [inp.tile([128, NK], f32, tag=f"val{t}", name=f"val{t}") for t in range(2)]
    wkt_sb = [inp.tile([128, CO], f32, tag=f"wkt{t}", name=f"wkt{t}") for t in range(2)]
    wqt_sb = [inp.tile([128, CO], f32, tag=f"wqt{t}", name=f"wqt{t}") for t in range(2)]
    bqk_sb = consts.tile([CO, 1], f32, tag="bqk")
    wf32p_sb = consts.tile([128, NS * NG], bf16, tag="wf32p")
    bf2_sb = consts.tile([128, 1], f32, tag="bf2")
    ident = consts.tile([128, 128], f32, tag="ident")
    ident_bf = consts.tile([NG, NG], bf16, tag="ident_bf")
    for t in range(2):
        sl = slice(t * 128, (t + 1) * 128)
        nc.sync.dma_start(out=key_sb[t][:], in_=keyb.ap()[sl, :])
        nc.sync.dma_start(out=qry_sb[t][:], in_=qryb.ap()[sl, :])
        nc.sync.dma_start(out=val_sb[t][:], in_=valb.ap()[sl, :])
        nc.sync.dma_start(out=wkt_sb[t][:], in_=wkt.ap()[sl, :])
        nc.sync.dma_start(out=wqt_sb[t][:], in_=wqt.ap()[sl, :])
    nc.sync.dma_start(out=bqk_sb[:], in_=bqk.ap())
    nc.sync.dma_start(out=wf32p_sb[:], in_=wf32p.ap())
    nc.sync.dma_start(out=bf2_sb[:], in_=bf2.ap())
    make_identity(nc, ident[:])
    make_identity(nc, ident_bf[:])

    ksbig = work.tile([128, NS * NK], bf16, tag="ksbig")
    qb_big = work.tile([128, NGRP * NS], f32, tag="qb_big")
    k2n_sb = work.tile([CO, NK], bf16, tag="k2n")
    qn_sb = work.tile([CO, NQ], f32, tag="qn")
    vt_sb = [
        work.tile([KT_SIZES[kt], 2 * 128], bf16, tag=f"vt{kt}", name=f"vt{kt}")
        for kt in range(5)
    ]
    attn_sb = [
        work.tile([KT_SIZES[kt], NQ], bf16, tag=f"attn{kt}", name=f"attn{kt}")
        for kt in range(5)
    ]

    with tc.tile_pool(name="ppro", bufs=2, space="PSUM") as ppro:
        # ---- k_ = WkT^T @ key -> (64, 576) bf16, then to DRAM scratch ----
        for half in range(2):
            pk2 = ppro.tile([CO, NQ], f32, tag="ppro")
            csl = slice(half * NQ, (half + 1) * NQ)
            for ct in range(2):
                nc.tensor.matmul(
                    out=pk2[:],
                    lhsT=wkt_sb[ct][:],
                    rhs=key_sb[ct][:, csl],
                    start=(ct == 0),
                    stop=(ct == 1),
                )
            nc.vector.tensor_copy(out=k2n_sb[:, csl], in_=pk2[:])
        nc.sync.dma_start(out=k2scr.ap(), in_=k2n_sb[:])

        # ---- q_ + bq + bk -> (64, 288) f32 ----
        pqn = ppro.tile([CO, NQ], f32, tag="ppro")
        for ct in range(2):
            nc.tensor.matmul(
                out=pqn[:],
                lhsT=wqt_sb[ct][:],
                rhs=qry_sb[ct][:],
                start=(ct == 0),
                stop=(ct == 1),
            )
        nc.vector.tensor_scalar_add(out=qn_sb[:], in0=pqn[:], scalar1=bqk_sb[:])

        # ---- replicate k rows: ksbig[32*rho + u, s*576 + k] = k_[16*rho + s, k]
        # (channel of (band rho, step s) is c = 16*rho + s, block-contiguous,
        # so each band's source is a plain row range of k2scr; the gpsimd
        # queue issues these so the SP queue stays free for qb scatter) ----
        for rho in range(RP):
            for sh in range(2):
                nsh = NS // 2
                dst = ksbig[
                    NG * rho : NG * (rho + 1), sh * nsh * NK : (sh + 1) * nsh * NK
                ].rearrange("p (s k) -> p s k", k=NK)
                src = (
                    k2scr.ap()[NS * rho + sh * nsh : NS * rho + (sh + 1) * nsh, :]
                    .rearrange("s k -> () s k")
                    .broadcast_to((NG, nsh, NK))
                )
                nc.gpsimd.dma_start(out=dst, in_=src)

        # ---- qRT = q_^T (q-part, c-free) via PE transpose ----
        qrt = work.tile([128, 3 * CO], f32, tag="qrt")
        for t in range(3):
            qsz = 128 if t < 2 else 32
            pqt = ppro.tile([128, CO], f32, tag="ppro")
            nc.tensor.transpose(
                out=pqt[:qsz, :],
                in_=qn_sb[:, t * 128 : t * 128 + qsz],
                identity=ident[:CO, :CO],
            )
            nc.vector.tensor_copy(
                out=qrt[:qsz, t * CO : (t + 1) * CO], in_=pqt[:qsz, :]
            )

        # ---- qb_big[32*rho + u, 16*G + s] = q_[16*rho + s, 32*G + u]
        # = qrt[32*(G%4) + u, 64*(G//4) + 16*rho + s]; 16 SBUF->SBUF DMAs ----
        for rho in range(RP):
            for w in range(4):
                nt = 3 if w == 0 else 2  # G = 4t + w must stay < 9
                dst = qb_big[
                    NG * rho : NG * (rho + 1), 16 * w : 16 * w + 64 * (nt - 1) + 16
                ].rearrange("p (t s) -> p t s", s=NS)[:, :: RP, :]
                src = qrt[
                    NG * w : NG * (w + 1), 16 * rho : 16 * rho + 64 * (nt - 1) + 16
                ].rearrange("p (t s) -> p t s", s=NS)[:, :: RP, :]
                nc.sync.dma_start(out=dst, in_=src)

        # ---- value transpose -> vT (k, cv) bf16 tiles ----
        for kt in range(5):
            pvt = ppro.tile([KT_SIZES[kt], 2 * 128], f32, tag="ppro")
            ks = slice(kt * 128, kt * 128 + KT_SIZES[kt])
            for ct in range(2):
                nc.tensor.transpose(
                    out=pvt[:, ct * 128 : (ct + 1) * 128],
                    in_=val_sb[ct][:, ks],
                    identity=ident[:],
                )
            nc.vector.tensor_copy(out=vt_sb[kt][:], in_=pvt[:])

    # ---- main loop over 9 q-groups ----
    with (
        tc.tile_pool(name="pre", bufs=4) as prep,
        tc.tile_pool(name="aq", bufs=2) as aqp,
        tc.tile_pool(name="scp", bufs=2, space="PSUM") as scp,
        tc.tile_pool(name="patt", bufs=2, space="PSUM") as pattp,
    ):
        SQ = 4  # s-steps per tanh chunk: keeps PE fed every ~2us (HAM warm)
        for G in range(NGRP):
            scg = scp.tile([NG, NK], f32, tag="scg")
            for sq in range(NS // SQ):
                pre = prep.tile([128, SQ * NK], bf16, tag="pre")
                for i in range(SQ):
                    s = sq * SQ + i
                    nc.vector.tensor_scalar_add(
                        out=pre[:, i * NK : (i + 1) * NK],
                        in0=ksbig[:, s * NK : (s + 1) * NK],
                        scalar1=qb_big[:, NS * G + s : NS * G + s + 1],
                    )
                nc.scalar.activation(pre[:], pre[:], AF.Tanh)
                for i in range(SQ):
                    s = sq * SQ + i
                    for c0, c1 in ((0, 512), (512, NK)):
                        nc.tensor.matmul(
                            out=scg[:, c0:c1],
                            lhsT=wf32p_sb[:, NG * s : NG * (s + 1)],
                            rhs=pre[:, i * NK + c0 : i * NK + c1],
                            start=(s == 0),
                            stop=(s == NS - 1),
                        )
            attn_q = aqp.tile([NG, NK], bf16, tag="attn_q")
            nc.scalar.activation(
                attn_q[:], scg[:], AF.Sigmoid, bias=bf2_sb[:NG, :]
            )
            for kt in range(5):
                ks = slice(kt * 128, kt * 128 + KT_SIZES[kt])
                patt = pattp.tile([KT_SIZES[kt], NG], bf16, tag="patt")
                nc.tensor.transpose(
                    out=patt[:], in_=attn_q[:, ks], identity=ident_bf[:]
                )
                nc.vector.tensor_copy(
                    out=attn_sb[kt][:, NG * G : NG * (G + 1)], in_=patt[:]
                )

    # ---- out = value @ attn : (256, 288) ----
    with tc.tile_pool(name="pout", bufs=2, space="PSUM") as pout:
        for ct in range(2):
            po = pout.tile([128, NQ], f32, tag="pout")
            for kt in range(5):
                nc.tensor.matmul(
                    out=po[:],
                    lhsT=vt_sb[kt][:, ct * 128 : (ct + 1) * 128],
                    rhs=attn_sb[kt][:],
                    start=(kt == 0),
                    stop=(kt == 4),
                )
            o_sb = work.tile([128, NQ], f32, tag=f"osb{ct}", name=f"osb{ct}")
            nc.vector.tensor_copy(out=o_sb[:], in_=po[:])
            nc.sync.dma_start(out=out.ap()[ct * 128 : (ct + 1) * 128, :], in_=o_sb[:])


def _build_pair(nc, mybir, tc, consts, inp, work, mode, chunk):
    """Older 2x64 pair layouts: mode 'bias' (ACT bias adds) or 'dve'."""
    from concourse.masks import make_identity

    f32 = mybir.dt.float32
    bf16 = mybir.dt.bfloat16
    AF = mybir.ActivationFunctionType

    keyb = nc.dram_tensor("keyb", [C, NK], f32, kind="ExternalInput")
    qryb = nc.dram_tensor("qryb", [C, NQ], f32, kind="ExternalInput")
    valb = nc.dram_tensor("valb", [C, NK], f32, kind="ExternalInput")
    wkt2 = nc.dram_tensor("wkt2", [C, 128], f32, kind="ExternalInput")
    wqt = nc.dram_tensor("wqt", [C, CO], f32, kind="ExternalInput")
    bqk2 = nc.dram_tensor("bqk2", [128, 1], f32, kind="ExternalInput")
    wf2 = nc.dram_tensor("wf2", [128, 2], bf16, kind="ExternalInput")
    bf2 = nc.dram_tensor("bf2", [128, 1], f32, kind="ExternalInput")
    out = nc.dram_tensor("out", [C, NQ], f32, kind="ExternalOutput")

    key_sb = [inp.tile([128, NK], f32, tag=f"key{t}", name=f"key{t}") for t in range(2)]
    qry_sb = [inp.tile([128, NQ], f32, tag=f"qry{t}", name=f"qry{t}") for t in range(2)]
    val_sb = [inp.tile([128, NK], f32, tag=f"val{t}", name=f"val{t}") for t in range(2)]
    wkt2_sb = [consts.tile([128, 128], f32, tag=f"wkt{t}", name=f"wkt{t}") for t in range(2)]
    wqt_sb = [consts.tile([128, CO], f32, tag=f"wqt{t}", name=f"wqt{t}") for t in range(2)]
    bqk2_sb = consts.tile([128, 1], f32, tag="bqk2")
    wf2_sb = consts.tile([128, 2], bf16, tag="wf2")
    bf2_sb = consts.tile([128, 1], f32, tag="bf2")
    ident = consts.tile([128, 128], f32, tag="ident")
    for t in range(2):
        sl = slice(t * 128, (t + 1) * 128)
        nc.sync.dma_start(out=key_sb[t][:], in_=keyb.ap()[sl, :])
        nc.sync.dma_start(out=qry_sb[t][:], in_=qryb.ap()[sl, :])
        nc.sync.dma_start(out=val_sb[t][:], in_=valb.ap()[sl, :])
        nc.sync.dma_start(out=wkt2_sb[t][:], in_=wkt2.ap()[sl, :])
        nc.sync.dma_start(out=wqt_sb[t][:], in_=wqt.ap()[sl, :])
    nc.sync.dma_start(out=bqk2_sb[:], in_=bqk2.ap())
    nc.sync.dma_start(out=wf2_sb[:], in_=wf2.ap())
    nc.sync.dma_start(out=bf2_sb[:], in_=bf2.ap())
    make_identity(nc, ident[:])

    with tc.tile_pool(name="ppro", bufs=1, space="PSUM") as ppro:
        k2_sb = work.tile([128, NK], bf16, tag="k2")
        for half in range(2):
            pk2 = ppro.tile([128, NQ], f32, tag="ppro")
            csl = slice(half * NQ, (half + 1) * NQ)
            for ct in range(2):
                nc.tensor.matmul(
                    out=pk2[:],
                    lhsT=wkt2_sb[ct][:],
                    rhs=key_sb[ct][:, csl],
                    start=(ct == 0),
                    stop=(ct == 1),
                )
            nc.vector.tensor_copy(out=k2_sb[:, csl], in_=pk2[:])

        pqb = ppro.tile([128, NPAIR], f32, tag="ppro")
        for par in range(2):
            for ct in range(2):
                nc.tensor.matmul(
                    out=pqb[par * CO : (par + 1) * CO, :],
                    lhsT=wqt_sb[ct][:],
                    rhs=qry_sb[ct][:, par : NQ : 2],
                    start=(ct == 0),
                    stop=(ct == 1),
                )
        qbias = work.tile([128, NPAIR], f32, tag="qbias")
        nc.vector.tensor_scalar_add(out=qbias[:], in0=pqb[:], scalar1=bqk2_sb[:])

        vt_sb = [
            work.tile([KT_SIZES[kt], 2 * 128], bf16, tag=f"vt{kt}", name=f"vt{kt}")
            for kt in range(5)
        ]
        for kt in range(5):
            pvt = ppro.tile([KT_SIZES[kt], 2 * 128], f32, tag="ppro")
            ks = slice(kt * 128, kt * 128 + KT_SIZES[kt])
            for ct in range(2):
                nc.tensor.transpose(
                    out=pvt[:, ct * 128 : (ct + 1) * 128],
                    in_=val_sb[ct][:, ks],
                    identity=ident[:],
                )
            nc.vector.tensor_copy(out=vt_sb[kt][:], in_=pvt[:])

    attn_sb = [
        work.tile([KT_SIZES[kt], NQ], bf16, tag=f"attn{kt}", name=f"attn{kt}")
        for kt in range(5)
    ]

    with tc.tile_pool(name="psc", bufs=1, space="PSUM") as psc:
        psc_t = [
            psc.tile([KT_SIZES[kt], NQ], f32, tag=f"sc{kt}", name=f"sc{kt}")
            for kt in range(5)
        ]
        if mode == "bias":
            with tc.tile_pool(name="pre", bufs=3) as prep:
                for j in range(NPAIR):
                    pre2 = prep.tile([128, NK], bf16, tag="pre2")
                    nc.scalar.activation(
                        pre2[:], k2_sb[:], AF.Tanh, bias=qbias[:, j : j + 1]
                    )
                    for kt in range(5):
                        ks = slice(kt * 128, kt * 128 + KT_SIZES[kt])
                        nc.tensor.matmul(
                            out=psc_t[kt][:, 2 * j : 2 * j + 2],
                            lhsT=pre2[:, ks],
                            rhs=wf2_sb[:],
                            start=True,
                            stop=True,
                        )
        else:  # dve
            nchunk = (NPAIR + chunk - 1) // chunk
            with tc.tile_pool(name="pre", bufs=2) as prep:
                for cidx in range(nchunk):
                    j0 = cidx * chunk
                    j1 = min(j0 + chunk, NPAIR)
                    pre = prep.tile([128, chunk * NK], bf16, tag="pre")
                    for j in range(j0, j1):
                        sl = slice((j - j0) * NK, (j - j0 + 1) * NK)
                        nc.vector.tensor_scalar_add(
                            out=pre[:, sl], in0=k2_sb[:], scalar1=qbias[:, j : j + 1]
                        )
                    nc.scalar.activation(
                        pre[:, : (j1 - j0) * NK], pre[:, : (j1 - j0) * NK], AF.Tanh
                    )
                    for j in range(j0, j1):
                        for kt in range(5):
                            ks = slice(
                                (j - j0) * NK + kt * 128,
                                (j - j0) * NK + kt * 128 + KT_SIZES[kt],
                            )
                            nc.tensor.matmul(
                                out=psc_t[kt][:, 2 * j : 2 * j + 2],
                                lhsT=pre[:, ks],
                                rhs=wf2_sb[:],
                                start=True,
                                stop=True,
                            )
        for kt in range(5):
            nc.scalar.activation(
                attn_sb[kt][:],
                psc_t[kt][:],
                AF.Sigmoid,
                bias=bf2_sb[: KT_SIZES[kt], :],
            )

    with tc.tile_pool(name="pout", bufs=2, space="PSUM") as pout:
        for ct in range(2):
            po = pout.tile([128, NQ], f32, tag="pout")
            for kt in range(5):
                nc.tensor.matmul(
                    out=po[:],
                    lhsT=vt_sb[kt][:, ct * 128 : (ct + 1) * 128],
                    rhs=attn_sb[kt][:],
                    start=(kt == 0),
                    stop=(kt == 4),
                )
            o_sb = work.tile([128, NQ], f32, tag=f"osb{ct}", name=f"osb{ct}")
            nc.vector.tensor_copy(out=o_sb[:], in_=po[:])
            nc.sync.dma_start(out=out.ap()[ct * 128 : (ct + 1) * 128, :], in_=o_sb[:])


def _build(mode="rep", chunk=12):
    import concourse.bacc as bacc
    import concourse.mybir as mybir
    from concourse.tile import TileContext

    nc = bacc.Bacc("TRN2", target_bir_lowering=False, debug=False, num_devices=8)
    with TileContext(nc) as tc:
        with (
            tc.tile_pool(name="consts", bufs=1) as consts,
            tc.tile_pool(name="inp", bufs=1) as inp,
            tc.tile_pool(name="work", bufs=1) as work,
        ):
            if mode == "rep":
                _build_rep(nc, mybir, tc, consts, inp, work)
            else:
                _build_pair(nc, mybir, tc, consts, inp, work, mode, chunk)
    nc.finalize()
    return nc


def _prep_in_maps(mode, key, query, value, Wk, bk, Wq, bq, wf, bf):
    import ml_dtypes

    f32 = np.float32
    key = np.ascontiguousarray(key, f32).reshape(B, C, NK)
    query = np.ascontiguousarray(query, f32).reshape(B, C, HW, HW)
    value = np.ascontiguousarray(value, f32).reshape(B, C, NK)
    WqT = np.ascontiguousarray(np.asarray(Wq, f32).T)  # (256, 64)
    bf2 = np.full((128, 1), np.float32(bf), f32)
    wf = np.asarray(wf, f32)

    common = {"wqt": WqT, "bf2": bf2}
    if mode == "rep":
        common["wkt"] = np.ascontiguousarray(np.asarray(Wk, f32).T)
        common["bqk"] = (np.asarray(bk, f32) + np.asarray(bq, f32)).reshape(CO, 1)
        wf32p = np.zeros((128, NS, NG), f32)
        for rho in range(RP):
            for s in range(NS):
                # channel of (band rho, step s) is 16*rho + s (block-contiguous)
                wf32p[NG * rho : NG * (rho + 1), s, :] = np.eye(NG, dtype=f32) * wf[
                    NS * rho + s
                ]
        common["wf32p"] = np.ascontiguousarray(
            wf32p.reshape(128, NS * NG).astype(ml_dtypes.bfloat16)
        )
    else:
        common["wkt2"] = np.ascontiguousarray(
            np.concatenate([np.asarray(Wk, f32).T] * 2, axis=1)
        )
        common["bqk2"] = np.ascontiguousarray(
            np.tile(np.asarray(bk, f32) + np.asarray(bq, f32), 2).reshape(128, 1)
        )
        wf2 = np.zeros((128, 2), f32)
        wf2[:CO, 0] = wf
        wf2[CO:, 1] = wf
        common["wf2"] = wf2.astype(ml_dtypes.bfloat16)

    in_maps = []
    for i in range(8):
        b, h = i // 2, i % 2
        qs = np.ascontiguousarray(query[b, :, h * 12 : (h + 1) * 12, :]).reshape(C, NQ)
        m = {"keyb": key[b], "qryb": qs, "valb": value[b]}
        m.update(common)
        in_maps.append(m)
    return in_maps


def run(mode="rep", chunk=12, trace=False, **inputs):
    from concourse.bass_utils import run_bass_kernel_spmd

    cache_key = (mode, chunk)
    if cache_key not in _cache:
        _cache[cache_key] = _build(mode, chunk)
    nc = _cache[cache_key]
    in_maps = _prep_in_maps(mode, **inputs)
    res = run_bass_kernel_spmd(nc, in_maps, core_ids=list(range(8)), trace=trace)
    out = np.empty((B, C, HW, HW), np.float32)
    for i in range(8):
        b, h = i // 2, i % 2
        out[b, :, h * 12 : (h + 1) * 12, :] = res.results[i]["out"].reshape(C, 12, HW)
    return out, res


def kernel(**inputs):
    out, _ = run(mode="rep", **inputs)
    return out


# revision 20
# speedup vs baseline: 1.1412x; 1.0091x over previous
"""Additive attention (B=4, C=256, CO=64, H=W=24) on 8 TRN2 NeuronCores.

Sharding: core i handles batch b = i // 2 and Nq-half h = i % 2 (rows
12h..12h+12 of the 24x24 query grid). Each core produces a complete
(256, 288) slice of the output; no collectives are needed.

Per-core math (Nk=576, Nq=288, CO=64):
  k_ = Wk @ key_b   (64, 576);  q_ = Wq @ qry_bh  (64, 288)
  scores[k, q] = sum_c wf[c] * tanh(k_[c, k] + q_[c, q] + bk[c] + bq[c]) + bf
  attn = sigmoid(scores);  out = value_b @ attn -> (256, 288)

"rep" layout (the fast path): partitions = 4 channel-rows x 32 q's
(c-major): partition p = 32*rho + u holds channel c = 4*s + rho for query
q = 32*G + u at channel-step s. The k_ rows are replicated 32x across
partitions via a DRAM round-trip DMA, the q_ column enters as the
per-partition scalar of a vector-engine add, tanh runs in big ACT
instructions, and a block-diagonal wf stationary reduces channels with
16 accumulating matmuls per query group -- scores land compact
(32 q, 576 k) in PSUM with full partition utilization everywhere.
"""

import numpy as np

B, C, CO, HW, NK = 4, 256, 64, 24, 576
NQ = 288  # per-core query count (half of 576)
NPAIR = NQ // 2
KT_SIZES = [128, 128, 128, 128, 64]  # 576 split into partition tiles
RP = 4  # channel rows per partition block ("rep" mode)
NG = 32  # q's per group
NS = CO // RP  # 16 channel steps
NGRP = NQ // NG  # 9 q groups

_cache = {}


def _build_rep(nc, mybir, tc, consts, inp, work):
    from concourse.masks import make_identity

    f32 = mybir.dt.float32
    bf16 = mybir.dt.bfloat16
    AF = mybir.ActivationFunctionType

    keyb = nc.dram_tensor("keyb", [C, NK], f32, kind="ExternalInput")
    qryb = nc.dram_tensor("qryb", [C, NQ], f32, kind="ExternalInput")
    valb = nc.dram_tensor("valb", [C, NK], f32, kind="ExternalInput")
    wkt = nc.dram_tensor("wkt", [C, CO], f32, kind="ExternalInput")
    wqt = nc.dram_tensor("wqt", [C, CO], f32, kind="ExternalInput")
    bqk = nc.dram_tensor("bqk", [CO, 1], f32, kind="ExternalInput")
    wf32p = nc.dram_tensor("wf32p", [128, NS * NG], bf16, kind="ExternalInput")
    bf2 = nc.dram_tensor("bf2", [128, 1], f32, kind="ExternalInput")
    out = nc.dram_tensor("out", [C, NQ], f32, kind="ExternalOutput")
    k2scr = nc.dram_tensor("k2scr", [CO, NK], bf16, kind="Internal")

    # ---- DMA inputs ----
    key_sb = [inp.tile([128, NK], f32, tag=f"key{t}", name=f"key{t}") for t in range(2)]
    qry_sb = [inp.tile([128, NQ], f32, tag=f"qry{t}", name=f"qry{t}") for t in range(2)]
    val_sb = [inp.tile([128, NK], f32, tag=f"val{t}", name=f"val{t}") for t in range(2)]
    wkt_sb = [inp.tile([128, CO], f32, tag=f"wkt{t}", name=f"wkt{t}") for t in range(2)]
    wqt_sb = [inp.tile([128, CO], f32, tag=f"wqt{t}", name=f"wqt{t}") for t in range(2)]
    bqk_sb = consts.tile([CO, 1], f32, tag="bqk")
    wf32p_sb = consts.tile([128, NS * NG], bf16, tag="wf32p")
    bf2_sb = consts.tile([128, 1], f32, tag="bf2")
    ident = consts.tile([128, 128], f32, tag="ident")
    ident_bf = consts.tile([NG, NG], bf16, tag="ident_bf")
    # critical-path inputs on the SP queue first; bulky value + consts on
    # the ACT/gpsimd queues so the k/q pipelines start ASAP
    for t in range(2):
        sl = slice(t * 128, (t + 1) * 128)
        nc.sync.dma_start(out=key_sb[t][:], in_=keyb.ap()[sl, :])
        nc.sync.dma_start(out=wkt_sb[t][:], in_=wkt.ap()[sl, :])
        nc.sync.dma_start(out=qry_sb[t][:], in_=qryb.ap()[sl, :])
        nc.sync.dma_start(out=wqt_sb[t][:], in_=wqt.ap()[sl, :])
    nc.scalar.dma_start(out=bqk_sb[:], in_=bqk.ap())
    nc.scalar.dma_start(out=wf32p_sb[:], in_=wf32p.ap())
    nc.scalar.dma_start(out=bf2_sb[:], in_=bf2.ap())
    for t in range(2):
        sl = slice(t * 128, (t + 1) * 128)
        nc.gpsimd.dma_start(out=val_sb[t][:], in_=valb.ap()[sl, :])
    make_identity(nc, ident[:])
    make_identity(nc, ident_bf[:])

    ksbig = work.tile([128, NS * NK], bf16, tag="ksbig")
    qb_big = work.tile([128, NGRP * NS], f32, tag="qb_big")
    k2n_sb = work.tile([CO, NK], bf16, tag="k2n")
    qn_sb = work.tile([CO, NQ], f32, tag="qn")
    vt_sb = [
        work.tile([KT_SIZES[kt], 2 * 128], bf16, tag=f"vt{kt}", name=f"vt{kt}")
        for kt in range(5)
    ]
    attn_sb = [
        work.tile([KT_SIZES[kt], NQ], bf16, tag=f"attn{kt}", name=f"attn{kt}")
        for kt in range(5)
    ]

    with tc.tile_pool(name="ppro", bufs=2, space="PSUM") as ppro:
        # ---- k_ = WkT^T @ key -> (64, 576) bf16, then to DRAM scratch ----
        for half in range(2):
            pk2 = ppro.tile([CO, NQ], f32, tag="ppro")
            csl = slice(half * NQ, (half + 1) * NQ)
            for ct in range(2):
                nc.tensor.matmul(
                    out=pk2[:],
                    lhsT=wkt_sb[ct][:],
                    rhs=key_sb[ct][:, csl],
                    start=(ct == 0),
                    stop=(ct == 1),
                )
            nc.vector.tensor_copy(out=k2n_sb[:, csl], in_=pk2[:])
        nc.scalar.dma_start(out=k2scr.ap(), in_=k2n_sb[:])

        # ---- q_ + bq + bk -> (64, 288) f32 ----
        pqn = ppro.tile([CO, NQ], f32, tag="ppro")
        for ct in range(2):
            nc.tensor.matmul(
                out=pqn[:],
                lhsT=wqt_sb[ct][:],
                rhs=qry_sb[ct][:],
                start=(ct == 0),
                stop=(ct == 1),
            )
        nc.vector.tensor_scalar_add(out=qn_sb[:], in0=pqn[:], scalar1=bqk_sb[:])

        # ---- replicate k rows: ksbig[32*rho + u, s*576 + k] = k_[16*rho + s, k]
        # (channel of (band rho, step s) is c = 16*rho + s, block-contiguous,
        # so each band's source is a plain row range of k2scr; the gpsimd
        # queue issues these so the SP queue stays free for qb scatter) ----
        for sh in range(2):  # s-half outer so early steps land first
            for rho in range(RP):
                nsh = NS // 2
                dst = ksbig[
                    NG * rho : NG * (rho + 1), sh * nsh * NK : (sh + 1) * nsh * NK
                ].rearrange("p (s k) -> p s k", k=NK)
                src = (
                    k2scr.ap()[NS * rho + sh * nsh : NS * rho + (sh + 1) * nsh, :]
                    .rearrange("s k -> () s k")
                    .broadcast_to((NG, nsh, NK))
                )
                (nc.scalar if rho % 2 else nc.sync).dma_start(out=dst, in_=src)

        # ---- qRT = q_^T (q-part, c-free) via PE transpose ----
        qrt = work.tile([128, 3 * CO], f32, tag="qrt")
        for t in range(3):
            qsz = 128 if t < 2 else 32
            pqt = ppro.tile([128, CO], f32, tag="ppro")
            nc.tensor.transpose(
                out=pqt[:qsz, :],
                in_=qn_sb[:, t * 128 : t * 128 + qsz],
                identity=ident[:CO, :CO],
            )
            nc.vector.tensor_copy(
                out=qrt[:qsz, t * CO : (t + 1) * CO], in_=pqt[:qsz, :]
            )

        # ---- qb_big[32*rho + u, 16*G + s] = q_[16*rho + s, 32*G + u]
        # = qrt[32*(G%4) + u, 64*(G//4) + 16*rho + s]; 16 SBUF->SBUF DMAs ----
        for w in range(4):  # w-outer so group 0's bias columns land first
            for rho in range(RP):
                nt = 3 if w == 0 else 2  # G = 4t + w must stay < 9
                dst = qb_big[
                    NG * rho : NG * (rho + 1), 16 * w : 16 * w + 64 * (nt - 1) + 16
                ].rearrange("p (t s) -> p t s", s=NS)[:, :: RP, :]
                src = qrt[
                    NG * w : NG * (w + 1), 16 * rho : 16 * rho + 64 * (nt - 1) + 16
                ].rearrange("p (t s) -> p t s", s=NS)[:, :: RP, :]
                (nc.scalar if rho % 2 else nc.sync).dma_start(out=dst, in_=src)

        # ---- value transpose -> vT (k, cv) bf16 tiles ----
        for kt in range(5):
            pvt = ppro.tile([KT_SIZES[kt], 2 * 128], f32, tag="ppro")
            ks = slice(kt * 128, kt * 128 + KT_SIZES[kt])
            for ct in range(2):
                nc.tensor.transpose(
                    out=pvt[:, ct * 128 : (ct + 1) * 128],
                    in_=val_sb[ct][:, ks],
                    identity=ident[:],
                )
            nc.vector.tensor_copy(out=vt_sb[kt][:], in_=pvt[:])

    # ---- main loop over 9 q-groups ----
    with (
        tc.tile_pool(name="pre", bufs=4) as prep,
        tc.tile_pool(name="aq", bufs=2) as aqp,
        tc.tile_pool(name="scp", bufs=2, space="PSUM") as scp,
        tc.tile_pool(name="patt", bufs=2, space="PSUM") as pattp,
    ):
        SQ = 4  # s-steps per tanh chunk: keeps PE fed every ~2us (HAM warm)
        for G in range(NGRP):
            scg = scp.tile([NG, NK], f32, tag="scg")
            for sq in range(NS // SQ):
                pre = prep.tile([128, SQ * NK], bf16, tag="pre")
                for i in range(SQ):
                    s = sq * SQ + i
                    nc.vector.tensor_scalar_add(
                        out=pre[:, i * NK : (i + 1) * NK],
                        in0=ksbig[:, s * NK : (s + 1) * NK],
                        scalar1=qb_big[:, NS * G + s : NS * G + s + 1],
                    )
                nc.scalar.activation(pre[:], pre[:], AF.Tanh)
                for i in range(SQ):
                    s = sq * SQ + i
                    for c0, c1 in ((0, 512), (512, NK)):
                        nc.tensor.matmul(
                            out=scg[:, c0:c1],
                            lhsT=wf32p_sb[:, NG * s : NG * (s + 1)],
                            rhs=pre[:, i * NK + c0 : i * NK + c1],
                            start=(s == 0),
                            stop=(s == NS - 1),
                        )
            attn_q = aqp.tile([NG, NK], bf16, tag="attn_q")
            nc.scalar.activation(
                attn_q[:], scg[:], AF.Sigmoid, bias=bf2_sb[:NG, :]
            )
            for kt in range(5):
                ks = slice(kt * 128, kt * 128 + KT_SIZES[kt])
                patt = pattp.tile([KT_SIZES[kt], NG], bf16, tag="patt")
                nc.tensor.transpose(
                    out=patt[:], in_=attn_q[:, ks], identity=ident_bf[:]
                )
                nc.vector.tensor_copy(
                    out=attn_sb[kt][:, NG * G : NG * (G + 1)], in_=patt[:]
                )

    # ---- out = value @ attn : (256, 288) ----
    with tc.tile_pool(name="pout", bufs=2, space="PSUM") as pout:
        for ct in range(2):
            po = pout.tile([128, NQ], f32, tag="pout")
            for kt in range(5):
                nc.tensor.matmul(
                    out=po[:],
                    lhsT=vt_sb[kt][:, ct * 128 : (ct + 1) * 128],
                    rhs=attn_sb[kt][:],
                    start=(kt == 0),
                    stop=(kt == 4),
                )
            o_sb = work.tile([128, NQ], f32, tag=f"osb{ct}", name=f"osb{ct}")
            nc.vector.tensor_copy(out=o_sb[:], in_=po[:])
            nc.sync.dma_start(out=out.ap()[ct * 128 : (ct + 1) * 128, :], in_=o_sb[:])


def _build_pair(nc, mybir, tc, consts, inp, work, mode, chunk):
    """Older 2x64 pair layouts: mode 'bias' (ACT bias adds) or 'dve'."""
    from concourse.masks import make_identity

    f32 = mybir.dt.float32
    bf16 = mybir.dt.bfloat16
    AF = mybir.ActivationFunctionType

    keyb = nc.dram_tensor("keyb", [C, NK], f32, kind="ExternalInput")
    qryb = nc.dram_tensor("qryb", [C, NQ], f32, kind="ExternalInput")
    valb = nc.dram_tensor("valb", [C, NK], f32, kind="ExternalInput")
    wkt2 = nc.dram_tensor("wkt2", [C, 128], f32, kind="ExternalInput")
    wqt = nc.dram_tensor("wqt", [C, CO], f32, kind="ExternalInput")
    bqk2 = nc.dram_tensor("bqk2", [128, 1], f32, kind="ExternalInput")
    wf2 = nc.dram_tensor("wf2", [128, 2], bf16, kind="ExternalInput")
    bf2 = nc.dram_tensor("bf2", [128, 1], f32, kind="ExternalInput")
    out = nc.dram_tensor("out", [C, NQ], f32, kind="ExternalOutput")

    key_sb = [inp.tile([128, NK], f32, tag=f"key{t}", name=f"key{t}") for t in range(2)]
    qry_sb = [inp.tile([128, NQ], f32, tag=f"qry{t}", name=f"qry{t}") for t in range(2)]
    val_sb = [inp.tile([128, NK], f32, tag=f"val{t}", name=f"val{t}") for t in range(2)]
    wkt2_sb = [consts.tile([128, 128], f32, tag=f"wkt{t}", name=f"wkt{t}") for t in range(2)]
    wqt_sb = [consts.tile([128, CO], f32, tag=f"wqt{t}", name=f"wqt{t}") for t in range(2)]
    bqk2_sb = consts.tile([128, 1], f32, tag="bqk2")
    wf2_sb = consts.tile([128, 2], bf16, tag="wf2")
    bf2_sb = consts.tile([128, 1], f32, tag="bf2")
    ident = consts.tile([128, 128], f32, tag="ident")
    for t in range(2):
        sl = slice(t * 128, (t + 1) * 128)
        nc.sync.dma_start(out=key_sb[t][:], in_=keyb.ap()[sl, :])
        nc.sync.dma_start(out=qry_sb[t][:], in_=qryb.ap()[sl, :])
        nc.sync.dma_start(out=val_sb[t][:], in_=valb.ap()[sl, :])
        nc.sync.dma_start(out=wkt2_sb[t][:], in_=wkt2.ap()[sl, :])
        nc.sync.dma_start(out=wqt_sb[t][:], in_=wqt.ap()[sl, :])
    nc.sync.dma_start(out=bqk2_sb[:], in_=bqk2.ap())
    nc.sync.dma_start(out=wf2_sb[:], in_=wf2.ap())
    nc.sync.dma_start(out=bf2_sb[:], in_=bf2.ap())
    make_identity(nc, ident[:])

    with tc.tile_pool(name="ppro", bufs=1, space="PSUM") as ppro:
        k2_sb = work.tile([128, NK], bf16, tag="k2")
        for half in range(2):
            pk2 = ppro.tile([128, NQ], f32, tag="ppro")
            csl = slice(half * NQ, (half + 1) * NQ)
            for ct in range(2):
                nc.tensor.matmul(
                    out=pk2[:],
                    lhsT=wkt2_sb[ct][:],
                    rhs=key_sb[ct][:, csl],
                    start=(ct == 0),
                    stop=(ct == 1),
                )
            nc.vector.tensor_copy(out=k2_sb[:, csl], in_=pk2[:])

        pqb = ppro.tile([128, NPAIR], f32, tag="ppro")
        for par in range(2):
            for ct in range(2):
                nc.tensor.matmul(
                    out=pqb[par * CO : (par + 1) * CO, :],
                    lhsT=wqt_sb[ct][:],
                    rhs=qry_sb[ct][:, par : NQ : 2],
                    start=(ct == 0),
                    stop=(ct == 1),
                )
        qbias = work.tile([128, NPAIR], f32, tag="qbias")
        nc.vector.tensor_scalar_add(out=qbias[:], in0=pqb[:], scalar1=bqk2_sb[:])

        vt_sb = [
            work.tile([KT_SIZES[kt], 2 * 128], bf16, tag=f"vt{kt}", name=f"vt{kt}")
            for kt in range(5)
        ]
        for kt in range(5):
            pvt = ppro.tile([KT_SIZES[kt], 2 * 128], f32, tag="ppro")
            ks = slice(kt * 128, kt * 128 + KT_SIZES[kt])
            for ct in range(2):
                nc.tensor.transpose(
                    out=pvt[:, ct * 128 : (ct + 1) * 128],
                    in_=val_sb[ct][:, ks],
                    identity=ident[:],
                )
            nc.vector.tensor_copy(out=vt_sb[kt][:], in_=pvt[:])

    attn_sb = [
        work.tile([KT_SIZES[kt], NQ], bf16, tag=f"attn{kt}", name=f"attn{kt}")
        for kt in range(5)
    ]

    with tc.tile_pool(name="psc", bufs=1, space="PSUM") as psc:
        psc_t = [
            psc.tile([KT_SIZES[kt], NQ], f32, tag=f"sc{kt}", name=f"sc{kt}")
            for kt in range(5)
        ]
        if mode == "bias":
            with tc.tile_pool(name="pre", bufs=3) as prep:
                for j in range(NPAIR):
                    pre2 = prep.tile([128, NK], bf16, tag="pre2")
                    nc.scalar.activation(
                        pre2[:], k2_sb[:], AF.Tanh, bias=qbias[:, j : j + 1]
                    )
                    for kt in range(5):
                        ks = slice(kt * 128, kt * 128 + KT_SIZES[kt])
                        nc.tensor.matmul(
                            out=psc_t[kt][:, 2 * j : 2 * j + 2],
                            lhsT=pre2[:, ks],
                            rhs=wf2_sb[:],
                            start=True,
                            stop=True,
                        )
        else:  # dve
            nchunk = (NPAIR + chunk - 1) // chunk
            with tc.tile_pool(name="pre", bufs=2) as prep:
                for cidx in range(nchunk):
                    j0 = cidx * chunk
                    j1 = min(j0 + chunk, NPAIR)
                    pre = prep.tile([128, chunk * NK], bf16, tag="pre")
                    for j in range(j0, j1):
                        sl = slice((j - j0) * NK, (j - j0 + 1) * NK)
                        nc.vector.tensor_scalar_add(
                            out=pre[:, sl], in0=k2_sb[:], scalar1=qbias[:, j : j + 1]
                        )
                    nc.scalar.activation(
                        pre[:, : (j1 - j0) * NK], pre[:, : (j1 - j0) * NK], AF.Tanh
                    )
                    for j in range(j0, j1):
                        for kt in range(5):
                            ks = slice(
                                (j - j0) * NK + kt * 128,
                                (j - j0) * NK + kt * 128 + KT_SIZES[kt],
                            )
                            nc.tensor.matmul(
                                out=psc_t[kt][:, 2 * j : 2 * j + 2],
                                lhsT=pre[:, ks],
                                rhs=wf2_sb[:],
                                start=True,
                                stop=True,
                            )
        for kt in range(5):
            nc.scalar.activation(
                attn_sb[kt][:],
                psc_t[kt][:],
                AF.Sigmoid,
                bias=bf2_sb[: KT_SIZES[kt], :],
            )

    with tc.tile_pool(name="pout", bufs=2, space="PSUM") as pout:
        for ct in range(2):
            po = pout.tile([128, NQ], f32, tag="pout")
            for kt in range(5):
                nc.tensor.matmul(
                    out=po[:],
                    lhsT=vt_sb[kt][:, ct * 128 : (ct + 1) * 128],
                    rhs=attn_sb[kt][:],
                    start=(kt == 0),
                    stop=(kt == 4),
                )
            o_sb = work.tile([128, NQ], f32, tag=f"osb{ct}", name=f"osb{ct}")
            nc.vector.tensor_copy(out=o_sb[:], in_=po[:])
            nc.sync.dma_start(out=out.ap()[ct * 128 : (ct + 1) * 128, :], in_=o_sb[:])


def _build(mode="rep", chunk=12):
    import concourse.bacc as bacc
    import concourse.mybir as mybir
    from concourse.tile import TileContext

    nc = bacc.Bacc("TRN2", target_bir_lowering=False, debug=False, num_devices=8)
    with TileContext(nc) as tc:
        with (
            tc.tile_pool(name="consts", bufs=1) as consts,
            tc.tile_pool(name="inp", bufs=1) as inp,
            tc.tile_pool(name="work", bufs=1) as work,
        ):
            if mode == "rep":
                _build_rep(nc, mybir, tc, consts, inp, work)
            else:
                _build_pair(nc, mybir, tc, consts, inp, work, mode, chunk)
    nc.finalize()
    return nc


def _prep_in_maps(mode, key, query, value, Wk, bk, Wq, bq, wf, bf):
    import ml_dtypes

    f32 = np.float32
    key = np.ascontiguousarray(key, f32).reshape(B, C, NK)
    query = np.ascontiguousarray(query, f32).reshape(B, C, HW, HW)
    value = np.ascontiguousarray(value, f32).reshape(B, C, NK)
    WqT = np.ascontiguousarray(np.asarray(Wq, f32).T)  # (256, 64)
    bf2 = np.full((128, 1), np.float32(bf), f32)
    wf = np.asarray(wf, f32)

    common = {"wqt": WqT, "bf2": bf2}
    if mode == "rep":
        common["wkt"] = np.ascontiguousarray(np.asarray(Wk, f32).T)
        common["bqk"] = (np.asarray(bk, f32) + np.asarray(bq, f32)).reshape(CO, 1)
        wf32p = np.zeros((128, NS, NG), f32)
        for rho in range(RP):
            for s in range(NS):
                # channel of (band rho, step s) is 16*rho + s (block-contiguous)
                wf32p[NG * rho : NG * (rho + 1), s, :] = np.eye(NG, dtype=f32) * wf[
                    NS * rho + s
                ]
        common["wf32p"] = np.ascontiguousarray(
            wf32p.reshape(128, NS * NG).astype(ml_dtypes.bfloat16)
        )
    else:
        common["wkt2"] = np.ascontiguousarray(
            np.concatenate([np.asarray(Wk, f32).T] * 2, axis=1)
        )
        common["bqk2"] = np.ascontiguousarray(
            np.tile(np.asarray(bk, f32) + np.asarray(bq, f32), 2).reshape(128, 1)
        )
        wf2 = np.zeros((128, 2), f32)
        wf2[:CO, 0] = wf
        wf2[CO:, 1] = wf
        common["wf2"] = wf2.astype(ml_dtypes.bfloat16)

    in_maps = []
    for i in range(8):
        b, h = i // 2, i % 2
        qs = np.ascontiguousarray(query[b, :, h * 12 : (h + 1) * 12, :]).reshape(C, NQ)
        m = {"keyb": key[b], "qryb": qs, "valb": value[b]}
        m.update(common)
        in_maps.append(m)
    return in_maps


def run(mode="rep", chunk=12, trace=False, **inputs):
    from concourse.bass_utils import run_bass_kernel_spmd

    cache_key = (mode, chunk)
    if cache_key not in _cache:
        _cache[cache_key] = _build(mode, chunk)
    nc = _cache[cache_key]
    in_maps = _prep_in_maps(mode, **inputs)
    res = run_bass_kernel_spmd(nc, in_maps, core_ids=list(range(8)), trace=trace)
    out = np.empty((B, C, HW, HW), np.float32)
    for i in range(8):
        b, h = i // 2, i % 2
        out[b, :, h * 12 : (h + 1) * 12, :] = res.results[i]["out"].reshape(C, 12, HW)
    return out, res


def kernel(**inputs):
    out, _ = run(mode="rep", **inputs)
    return out


# revision 24
# speedup vs baseline: 1.1751x; 1.0297x over previous
"""Additive attention (B=4, C=256, CO=64, H=W=24) on 8 TRN2 NeuronCores.

Sharding: core i handles batch b = i // 2 and Nq-half h = i % 2 (rows
12h..12h+12 of the 24x24 query grid). Each core produces a complete
(256, 288) slice of the output; no collectives are needed.

Per-core math (Nk=576, Nq=288, CO=64):
  k_ = Wk @ key_b   (64, 576);  q_ = Wq @ qry_bh  (64, 288)
  scores[k, q] = sum_c wf[c] * tanh(k_[c, k] + q_[c, q] + bk[c] + bq[c]) + bf
  attn = sigmoid(scores);  out = value_b @ attn -> (256, 288)

"rep" layout (the fast path): partitions = 4 channel-rows x 32 q's
(c-major): partition p = 32*rho + u holds channel c = 4*s + rho for query
q = 32*G + u at channel-step s. The k_ rows are replicated 32x across
partitions via a DRAM round-trip DMA, the q_ column enters as the
per-partition scalar of a vector-engine add, tanh runs in big ACT
instructions, and a block-diagonal wf stationary reduces channels with
16 accumulating matmuls per query group -- scores land compact
(32 q, 576 k) in PSUM with full partition utilization everywhere.
"""

import numpy as np

B, C, CO, HW, NK = 4, 256, 64, 24, 576
NQ = 288  # per-core query count (half of 576)
NPAIR = NQ // 2
KT_SIZES = [128, 128, 128, 128, 64]  # 576 split into partition tiles
RP = 4  # channel rows per partition block ("rep" mode)
NG = 32  # q's per group
NS = CO // RP  # 16 channel steps
NGRP = NQ // NG  # 9 q groups

_cache = {}


def _build_rep(nc, mybir, tc, consts, inp, work):
    from concourse.masks import make_identity

    f32 = mybir.dt.float32
    bf16 = mybir.dt.bfloat16
    AF = mybir.ActivationFunctionType

    keyb = nc.dram_tensor("keyb", [C, NK], bf16, kind="ExternalInput")
    qryb = nc.dram_tensor("qryb", [C, NQ], bf16, kind="ExternalInput")
    valb = nc.dram_tensor("valb", [C, NK], bf16, kind="ExternalInput")
    wkt = nc.dram_tensor("wkt", [C, CO], bf16, kind="ExternalInput")
    wqt = nc.dram_tensor("wqt", [C, CO], bf16, kind="ExternalInput")
    bqk = nc.dram_tensor("bqk", [CO, 1], f32, kind="ExternalInput")
    wf32p = nc.dram_tensor("wf32p", [128, NS * NG], bf16, kind="ExternalInput")
    bf2 = nc.dram_tensor("bf2", [128, 1], f32, kind="ExternalInput")
    out = nc.dram_tensor("out", [C, NQ], f32, kind="ExternalOutput")
    k2scr = nc.dram_tensor("k2scr", [CO, NK], bf16, kind="Internal")
    qrtscr = nc.dram_tensor("qrtscr", [NQ, CO], f32, kind="Internal")

    # ---- DMA inputs ----
    key_sb = [inp.tile([128, NK], bf16, tag=f"key{t}", name=f"key{t}") for t in range(2)]
    qry_sb = [inp.tile([128, NQ], bf16, tag=f"qry{t}", name=f"qry{t}") for t in range(2)]
    val_sb = [inp.tile([128, NK], bf16, tag=f"val{t}", name=f"val{t}") for t in range(2)]
    wkt_sb = [inp.tile([128, CO], bf16, tag=f"wkt{t}", name=f"wkt{t}") for t in range(2)]
    wqt_sb = [inp.tile([128, CO], bf16, tag=f"wqt{t}", name=f"wqt{t}") for t in range(2)]
    bqk_sb = consts.tile([CO, 1], f32, tag="bqk")
    wf32p_sb = consts.tile([128, NS * NG], bf16, tag="wf32p")
    bf2_sb = consts.tile([128, 1], f32, tag="bf2")
    ident = consts.tile([128, 128], f32, tag="ident")
    ident_bf = consts.tile([128, 128], bf16, tag="ident_bf")
    # critical-path inputs on the SP queue first; bulky value + consts on
    # the ACT/gpsimd queues so the k/q pipelines start ASAP
    for t in range(2):
        sl = slice(t * 128, (t + 1) * 128)
        nc.sync.dma_start(out=key_sb[t][:], in_=keyb.ap()[sl, :])
        nc.sync.dma_start(out=wkt_sb[t][:], in_=wkt.ap()[sl, :])
        nc.sync.dma_start(out=qry_sb[t][:], in_=qryb.ap()[sl, :])
        nc.sync.dma_start(out=wqt_sb[t][:], in_=wqt.ap()[sl, :])
    nc.scalar.dma_start(out=bqk_sb[:], in_=bqk.ap())
    nc.scalar.dma_start(out=wf32p_sb[:], in_=wf32p.ap())
    nc.scalar.dma_start(out=bf2_sb[:], in_=bf2.ap())
    for t in range(2):
        sl = slice(t * 128, (t + 1) * 128)
        nc.gpsimd.dma_start(out=val_sb[t][:], in_=valb.ap()[sl, :])
    make_identity(nc, ident[:])
    make_identity(nc, ident_bf[:])

    ksbig = work.tile([128, NS * NK], bf16, tag="ksbig")
    qb_big = work.tile([128, NGRP * NS], f32, tag="qb_big")
    k2n_sb = work.tile([CO, NK], bf16, tag="k2n")
    qn_sb = work.tile([CO, NQ], f32, tag="qn")
    vt_sb = [
        work.tile([KT_SIZES[kt], 2 * 128], bf16, tag=f"vt{kt}", name=f"vt{kt}")
        for kt in range(5)
    ]
    attn_sb = [
        work.tile([KT_SIZES[kt], NQ], bf16, tag=f"attn{kt}", name=f"attn{kt}")
        for kt in range(5)
    ]

    with tc.tile_pool(name="ppro", bufs=2, space="PSUM") as ppro:
        # ---- k_ = WkT^T @ key -> (64, 576) bf16, then to DRAM scratch ----
        for half in range(2):
            pk2 = ppro.tile([CO, NQ], f32, tag="ppro")
            csl = slice(half * NQ, (half + 1) * NQ)
            for ct in range(2):
                nc.tensor.matmul(
                    out=pk2[:],
                    lhsT=wkt_sb[ct][:],
                    rhs=key_sb[ct][:, csl],
                    start=(ct == 0),
                    stop=(ct == 1),
                )
            nc.vector.tensor_copy(out=k2n_sb[:, csl], in_=pk2[:])
        nc.scalar.dma_start(out=k2scr.ap(), in_=k2n_sb[:])

        # ---- q_ + bq + bk -> (64, 288) f32 ----
        pqn = ppro.tile([CO, NQ], f32, tag="ppro")
        for ct in range(2):
            nc.tensor.matmul(
                out=pqn[:],
                lhsT=wqt_sb[ct][:],
                rhs=qry_sb[ct][:],
                start=(ct == 0),
                stop=(ct == 1),
            )
        nc.vector.tensor_scalar_add(out=qn_sb[:], in0=pqn[:], scalar1=bqk_sb[:])

        # ---- replicate k rows: ksbig[32*rho + u, s*576 + k] = k_[16*rho + s, k]
        # (channel of (band rho, step s) is c = 16*rho + s, block-contiguous,
        # so each band's source is a plain row range of k2scr; the gpsimd
        # queue issues these so the SP queue stays free for qb scatter) ----
        for rho in range(RP):
            dst = ksbig[NG * rho : NG * (rho + 1), :].rearrange(
                "p (s k) -> p s k", k=NK
            )
            srcap = (
                k2scr.ap()[NS * rho : NS * (rho + 1), :]
                .rearrange("s k -> () s k")
                .broadcast_to((NG, NS, NK))
            )
            (nc.scalar if rho % 2 else nc.sync).dma_start(out=dst, in_=srcap)

        # ---- qRT = q_^T (q-part, c-free) via PE transpose ----
        qrt = work.tile([128, 3 * CO], f32, tag="qrt")
        for t in range(3):
            qsz = 128 if t < 2 else 32
            pqt = ppro.tile([128, CO], f32, tag="ppro")
            nc.tensor.transpose(
                out=pqt[:qsz, :],
                in_=qn_sb[:, t * 128 : t * 128 + qsz],
                identity=ident[:CO, :CO],
            )
            nc.vector.tensor_copy(
                out=qrt[:qsz, t * CO : (t + 1) * CO], in_=pqt[:qsz, :]
            )

        # ---- qb_big[32*rho + u, 16*G + s] = q_[16*rho + s, 32*G + u]
        # qrt -> DRAM (q, c) scratch (3 writes), then one clean strided read
        # per band: src[u, G, s] = qrtscr[32G + u, 16rho + s] ----
        for t in range(3):
            qsz = 128 if t < 2 else 32
            (nc.scalar if t % 2 else nc.sync).dma_start(
                out=qrtscr.ap()[t * 128 : t * 128 + qsz, :],
                in_=qrt[:qsz, t * CO : (t + 1) * CO],
            )
        for rho in range(RP):
            srcap = qrtscr.ap().rearrange("(g u) c -> u g c", u=NG)[
                :, :, 16 * rho : 16 * (rho + 1)
            ]
            (nc.scalar if rho % 2 else nc.sync).dma_start(
                out=qb_big[NG * rho : NG * (rho + 1), :].rearrange(
                    "p (g s) -> p g s", s=NS
                ),
                in_=srcap,
            )

        # ---- value transpose -> vT (k, cv) bf16 tiles ----
        for kt in range(5):
            pvt = ppro.tile([KT_SIZES[kt], 2 * 128], bf16, tag="pprobf")
            ks = slice(kt * 128, kt * 128 + KT_SIZES[kt])
            for ct in range(2):
                nc.tensor.transpose(
                    out=pvt[:, ct * 128 : (ct + 1) * 128],
                    in_=val_sb[ct][:, ks],
                    identity=ident_bf[:],
                )
            nc.vector.tensor_copy(out=vt_sb[kt][:], in_=pvt[:])

    # ---- main loop over 9 q-groups ----
    with (
        tc.tile_pool(name="pre", bufs=4) as prep,
        tc.tile_pool(name="aq", bufs=2) as aqp,
        tc.tile_pool(name="scp", bufs=2, space="PSUM") as scp,
        tc.tile_pool(name="patt", bufs=2, space="PSUM") as pattp,
    ):
        SQ = 4  # s-steps per tanh chunk: keeps PE fed every ~2us (HAM warm)
        for G in range(NGRP):
            scg = scp.tile([NG, NK], f32, tag="scg")
            for sq in range(NS // SQ):
                pre = prep.tile([128, SQ * NK], bf16, tag="pre")
                for i in range(SQ):
                    s = sq * SQ + i
                    nc.vector.tensor_scalar_add(
                        out=pre[:, i * NK : (i + 1) * NK],
                        in0=ksbig[:, s * NK : (s + 1) * NK],
                        scalar1=qb_big[:, NS * G + s : NS * G + s + 1],
                    )
                nc.scalar.activation(pre[:], pre[:], AF.Tanh)
                for i in range(SQ):
                    s = sq * SQ + i
                    for c0, c1 in ((0, 512), (512, NK)):
                        nc.tensor.matmul(
                            out=scg[:, c0:c1],
                            lhsT=wf32p_sb[:, NG * s : NG * (s + 1)],
                            rhs=pre[:, i * NK + c0 : i * NK + c1],
                            start=(s == 0),
                            stop=(s == NS - 1),
                        )
            attn_q = aqp.tile([NG, NK], bf16, tag="attn_q")
            nc.scalar.activation(
                attn_q[:], scg[:], AF.Sigmoid, bias=bf2_sb[:NG, :]
            )
            for kt in range(5):
                ks = slice(kt * 128, kt * 128 + KT_SIZES[kt])
                patt = pattp.tile([KT_SIZES[kt], NG], bf16, tag="patt")
                nc.tensor.transpose(
                    out=patt[:], in_=attn_q[:, ks], identity=ident_bf[:NG, :NG]
                )
                nc.vector.tensor_copy(
                    out=attn_sb[kt][:, NG * G : NG * (G + 1)], in_=patt[:]
                )

    # ---- out = value @ attn : (256, 288) ----
    with tc.tile_pool(name="pout", bufs=2, space="PSUM") as pout:
        for ct in range(2):
            po = pout.tile([128, NQ], f32, tag="pout")
            for kt in range(5):
                nc.tensor.matmul(
                    out=po[:],
                    lhsT=vt_sb[kt][:, ct * 128 : (ct + 1) * 128],
                    rhs=attn_sb[kt][:],
                    start=(kt == 0),
                    stop=(kt == 4),
                )
            o_sb = work.tile([128, NQ], f32, tag=f"osb{ct}", name=f"osb{ct}")
            nc.vector.tensor_copy(out=o_sb[:], in_=po[:])
            nc.sync.dma_start(out=out.ap()[ct * 128 : (ct + 1) * 128, :], in_=o_sb[:])


def _build_pair(nc, mybir, tc, consts, inp, work, mode, chunk):
    """Older 2x64 pair layouts: mode 'bias' (ACT bias adds) or 'dve'."""
    from concourse.masks import make_identity

    f32 = mybir.dt.float32
    bf16 = mybir.dt.bfloat16
    AF = mybir.ActivationFunctionType

    keyb = nc.dram_tensor("keyb", [C, NK], f32, kind="ExternalInput")
    qryb = nc.dram_tensor("qryb", [C, NQ], f32, kind="ExternalInput")
    valb = nc.dram_tensor("valb", [C, NK], f32, kind="ExternalInput")
    wkt2 = nc.dram_tensor("wkt2", [C, 128], f32, kind="ExternalInput")
    wqt = nc.dram_tensor("wqt", [C, CO], f32, kind="ExternalInput")
    bqk2 = nc.dram_tensor("bqk2", [128, 1], f32, kind="ExternalInput")
    wf2 = nc.dram_tensor("wf2", [128, 2], bf16, kind="ExternalInput")
    bf2 = nc.dram_tensor("bf2", [128, 1], f32, kind="ExternalInput")
    out = nc.dram_tensor("out", [C, NQ], f32, kind="ExternalOutput")

    key_sb = [inp.tile([128, NK], f32, tag=f"key{t}", name=f"key{t}") for t in range(2)]
    qry_sb = [inp.tile([128, NQ], f32, tag=f"qry{t}", name=f"qry{t}") for t in range(2)]
    val_sb = [inp.tile([128, NK], f32, tag=f"val{t}", name=f"val{t}") for t in range(2)]
    wkt2_sb = [consts.tile([128, 128], f32, tag=f"wkt{t}", name=f"wkt{t}") for t in range(2)]
    wqt_sb = [consts.tile([128, CO], f32, tag=f"wqt{t}", name=f"wqt{t}") for t in range(2)]
    bqk2_sb = consts.tile([128, 1], f32, tag="bqk2")
    wf2_sb = consts.tile([128, 2], bf16, tag="wf2")
    bf2_sb = consts.tile([128, 1], f32, tag="bf2")
    ident = consts.tile([128, 128], f32, tag="ident")
    for t in range(2):
        sl = slice(t * 128, (t + 1) * 128)
        nc.sync.dma_start(out=key_sb[t][:], in_=keyb.ap()[sl, :])
        nc.sync.dma_start(out=qry_sb[t][:], in_=qryb.ap()[sl, :])
        nc.sync.dma_start(out=val_sb[t][:], in_=valb.ap()[sl, :])
        nc.sync.dma_start(out=wkt2_sb[t][:], in_=wkt2.ap()[sl, :])
        nc.sync.dma_start(out=wqt_sb[t][:], in_=wqt.ap()[sl, :])
    nc.sync.dma_start(out=bqk2_sb[:], in_=bqk2.ap())
    nc.sync.dma_start(out=wf2_sb[:], in_=wf2.ap())
    nc.sync.dma_start(out=bf2_sb[:], in_=bf2.ap())
    make_identity(nc, ident[:])

    with tc.tile_pool(name="ppro", bufs=1, space="PSUM") as ppro:
        k2_sb = work.tile([128, NK], bf16, tag="k2")
        for half in range(2):
            pk2 = ppro.tile([128, NQ], f32, tag="ppro")
            csl = slice(half * NQ, (half + 1) * NQ)
            for ct in range(2):
                nc.tensor.matmul(
                    out=pk2[:],
                    lhsT=wkt2_sb[ct][:],
                    rhs=key_sb[ct][:, csl],
                    start=(ct == 0),
                    stop=(ct == 1),
                )
            nc.vector.tensor_copy(out=k2_sb[:, csl], in_=pk2[:])

        pqb = ppro.tile([128, NPAIR], f32, tag="ppro")
        for par in range(2):
            for ct in range(2):
                nc.tensor.matmul(
                    out=pqb[par * CO : (par + 1) * CO, :],
                    lhsT=wqt_sb[ct][:],
                    rhs=qry_sb[ct][:, par : NQ : 2],
                    start=(ct == 0),
                    stop=(ct == 1),
                )
        qbias = work.tile([128, NPAIR], f32, tag="qbias")
        nc.vector.tensor_scalar_add(out=qbias[:], in0=pqb[:], scalar1=bqk2_sb[:])

        vt_sb = [
            work.tile([KT_SIZES[kt], 2 * 128], bf16, tag=f"vt{kt}", name=f"vt{kt}")
            for kt in range(5)
        ]
        for kt in range(5):
            pvt = ppro.tile([KT_SIZES[kt], 2 * 128], bf16, tag="pprobf")
            ks = slice(kt * 128, kt * 128 + KT_SIZES[kt])
            for ct in range(2):
                nc.tensor.transpose(
                    out=pvt[:, ct * 128 : (ct + 1) * 128],
                    in_=val_sb[ct][:, ks],
                    identity=ident_bf[:],
                )
            nc.vector.tensor_copy(out=vt_sb[kt][:], in_=pvt[:])

    attn_sb = [
        work.tile([KT_SIZES[kt], NQ], bf16, tag=f"attn{kt}", name=f"attn{kt}")
        for kt in range(5)
    ]

    with tc.tile_pool(name="psc", bufs=1, space="PSUM") as psc:
        psc_t = [
            psc.tile([KT_SIZES[kt], NQ], f32, tag=f"sc{kt}", name=f"sc{kt}")
            for kt in range(5)
        ]
        if mode == "bias":
            with tc.tile_pool(name="pre", bufs=3) as prep:
                for j in range(NPAIR):
                    pre2 = prep.tile([128, NK], bf16, tag="pre2")
                    nc.scalar.activation(
                        pre2[:], k2_sb[:], AF.Tanh, bias=qbias[:, j : j + 1]
                    )
                    for kt in range(5):
                        ks = slice(kt * 128, kt * 128 + KT_SIZES[kt])
                        nc.tensor.matmul(
                            out=psc_t[kt][:, 2 * j : 2 * j + 2],
                            lhsT=pre2[:, ks],
                            rhs=wf2_sb[:],
                            start=True,
                            stop=True,
                        )
        else:  # dve
            nchunk = (NPAIR + chunk - 1) // chunk
            with tc.tile_pool(name="pre", bufs=2) as prep:
                for cidx in range(nchunk):
                    j0 = cidx * chunk
                    j1 = min(j0 + chunk, NPAIR)
                    pre = prep.tile([128, chunk * NK], bf16, tag="pre")
                    for j in range(j0, j1):
                        sl = slice((j - j0) * NK, (j - j0 + 1) * NK)
                        nc.vector.tensor_scalar_add(
                            out=pre[:, sl], in0=k2_sb[:], scalar1=qbias[:, j : j + 1]
                        )
                    nc.scalar.activation(
                        pre[:, : (j1 - j0) * NK], pre[:, : (j1 - j0) * NK], AF.Tanh
                    )
                    for j in range(j0, j1):
                        for kt in range(5):
                            ks = slice(
                                (j - j0) * NK + kt * 128,
                                (j - j0) * NK + kt * 128 + KT_SIZES[kt],
                            )
                            nc.tensor.matmul(
                                out=psc_t[kt][:, 2 * j : 2 * j + 2],
                                lhsT=pre[:, ks],
                                rhs=wf2_sb[:],
                                start=True,
                                stop=True,
                            )
        for kt in range(5):
            nc.scalar.activation(
                attn_sb[kt][:],
                psc_t[kt][:],
                AF.Sigmoid,
                bias=bf2_sb[: KT_SIZES[kt], :],
            )

    with tc.tile_pool(name="pout", bufs=2, space="PSUM") as pout:
        for ct in range(2):
            po = pout.tile([128, NQ], f32, tag="pout")
            for kt in range(5):
                nc.tensor.matmul(
                    out=po[:],
                    lhsT=vt_sb[kt][:, ct * 128 : (ct + 1) * 128],
                    rhs=attn_sb[kt][:],
                    start=(kt == 0),
                    stop=(kt == 4),
                )
            o_sb = work.tile([128, NQ], f32, tag=f"osb{ct}", name=f"osb{ct}")
            nc.vector.tensor_copy(out=o_sb[:], in_=po[:])
            nc.sync.dma_start(out=out.ap()[ct * 128 : (ct + 1) * 128, :], in_=o_sb[:])


def _build(mode="rep", chunk=12):
    import concourse.bacc as bacc
    import concourse.mybir as mybir
    from concourse.tile import TileContext

    nc = bacc.Bacc("TRN2", target_bir_lowering=False, debug=False, num_devices=8)
    with TileContext(nc) as tc:
        with (
            tc.tile_pool(name="consts", bufs=1) as consts,
            tc.tile_pool(name="inp", bufs=1) as inp,
            tc.tile_pool(name="work", bufs=1) as work,
        ):
            if mode == "rep":
                _build_rep(nc, mybir, tc, consts, inp, work)
            else:
                _build_pair(nc, mybir, tc, consts, inp, work, mode, chunk)
    nc.finalize()
    return nc


def _prep_in_maps(mode, key, query, value, Wk, bk, Wq, bq, wf, bf):
    import ml_dtypes

    f32 = np.float32
    key = np.ascontiguousarray(key, f32).reshape(B, C, NK)
    query = np.ascontiguousarray(query, f32).reshape(B, C, HW, HW)
    value = np.ascontiguousarray(value, f32).reshape(B, C, NK)
    WqT = np.ascontiguousarray(np.asarray(Wq, f32).T)  # (256, 64)
    bf2 = np.full((128, 1), np.float32(bf), f32)
    wf = np.asarray(wf, f32)

    common = {"wqt": WqT, "bf2": bf2}
    if mode == "rep":
        import ml_dtypes as mld

        common["wqt"] = WqT.astype(mld.bfloat16)
        common["wkt"] = np.ascontiguousarray(np.asarray(Wk, f32).T).astype(mld.bfloat16)
        common["bqk"] = (np.asarray(bk, f32) + np.asarray(bq, f32)).reshape(CO, 1)
        wf32p = np.zeros((128, NS, NG), f32)
        for rho in range(RP):
            for s in range(NS):
                # channel of (band rho, step s) is 16*rho + s (block-contiguous)
                wf32p[NG * rho : NG * (rho + 1), s, :] = np.eye(NG, dtype=f32) * wf[
                    NS * rho + s
                ]
        common["wf32p"] = np.ascontiguousarray(
            wf32p.reshape(128, NS * NG).astype(ml_dtypes.bfloat16)
        )
    else:
        common["wkt2"] = np.ascontiguousarray(
            np.concatenate([np.asarray(Wk, f32).T] * 2, axis=1)
        )
        common["bqk2"] = np.ascontiguousarray(
            np.tile(np.asarray(bk, f32) + np.asarray(bq, f32), 2).reshape(128, 1)
        )
        wf2 = np.zeros((128, 2), f32)
        wf2[:CO, 0] = wf
        wf2[CO:, 1] = wf
        common["wf2"] = wf2.astype(ml_dtypes.bfloat16)

    if mode == "rep":
        import ml_dtypes as mld

        key = key.astype(mld.bfloat16)
        query = query.astype(mld.bfloat16)
        value = value.astype(mld.bfloat16)
    in_maps = []
    for i in range(8):
        b, h = i // 2, i % 2
        qs = np.ascontiguousarray(query[b, :, h * 12 : (h + 1) * 12, :]).reshape(C, NQ)
        m = {"keyb": np.ascontiguousarray(key[b]), "qryb": qs, "valb": np.ascontiguousarray(value[b])}
        m.update(common)
        in_maps.append(m)
    return in_maps


def run(mode="rep", chunk=12, trace=False, **inputs):
    from concourse.bass_utils import run_bass_kernel_spmd

    cache_key = (mode, chunk)
    if cache_key not in _cache:
        _cache[cache_key] = _build(mode, chunk)
    nc = _cache[cache_key]
    in_maps = _prep_in_maps(mode, **inputs)
    res = run_bass_kernel_spmd(nc, in_maps, core_ids=list(range(8)), trace=trace)
    out = np.empty((B, C, HW, HW), np.float32)
    for i in range(8):
        b, h = i // 2, i % 2
        out[b, :, h * 12 : (h + 1) * 12, :] = res.results[i]["out"].reshape(C, 12, HW)
    return out, res


def kernel(**inputs):
    out, _ = run(mode="rep", **inputs)
    return out


# revision 26
# speedup vs baseline: 1.1911x; 1.0136x over previous
"""Additive attention (B=4, C=256, CO=64, H=W=24) on 8 TRN2 NeuronCores.

Sharding: core i handles batch b = i // 2 and Nq-half h = i % 2 (rows
12h..12h+12 of the 24x24 query grid). Each core produces a complete
(256, 288) slice of the output; no collectives are needed.

Per-core math (Nk=576, Nq=288, CO=64):
  k_ = Wk @ key_b   (64, 576);  q_ = Wq @ qry_bh  (64, 288)
  scores[k, q] = sum_c wf[c] * tanh(k_[c, k] + q_[c, q] + bk[c] + bq[c]) + bf
  attn = sigmoid(scores);  out = value_b @ attn -> (256, 288)

"rep" layout (the fast path): partitions = 4 channel-rows x 32 q's
(c-major): partition p = 32*rho + u holds channel c = 4*s + rho for query
q = 32*G + u at channel-step s. The k_ rows are replicated 32x across
partitions via a DRAM round-trip DMA, the q_ column enters as the
per-partition scalar of a vector-engine add, tanh runs in big ACT
instructions, and a block-diagonal wf stationary reduces channels with
16 accumulating matmuls per query group -- scores land compact
(32 q, 576 k) in PSUM with full partition utilization everywhere.
"""

import numpy as np

B, C, CO, HW, NK = 4, 256, 64, 24, 576
NQ = 288  # per-core query count (half of 576)
NPAIR = NQ // 2
KT_SIZES = [128, 128, 128, 128, 64]  # 576 split into partition tiles
RP = 4  # channel rows per partition block ("rep" mode)
NG = 32  # q's per group
NS = CO // RP  # 16 channel steps
NGRP = NQ // NG  # 9 q groups

_cache = {}


def _build_rep(nc, mybir, tc, consts, inp, work):
    from concourse.masks import make_identity

    f32 = mybir.dt.float32
    bf16 = mybir.dt.bfloat16
    AF = mybir.ActivationFunctionType

    keyb = nc.dram_tensor("keyb", [C, NK], bf16, kind="ExternalInput")
    qryb = nc.dram_tensor("qryb", [C, NQ], bf16, kind="ExternalInput")
    valb = nc.dram_tensor("valb", [C, NK], bf16, kind="ExternalInput")
    wkt = nc.dram_tensor("wkt", [C, CO], bf16, kind="ExternalInput")
    wqt = nc.dram_tensor("wqt", [C, CO], bf16, kind="ExternalInput")
    bqk = nc.dram_tensor("bqk", [CO, 1], f32, kind="ExternalInput")
    wf32p = nc.dram_tensor("wf32p", [128, NS * NG], bf16, kind="ExternalInput")
    bf2 = nc.dram_tensor("bf2", [128, 1], f32, kind="ExternalInput")
    out = nc.dram_tensor("out", [C, NQ], f32, kind="ExternalOutput")
    k2scr = nc.dram_tensor("k2scr", [CO, NK], bf16, kind="Internal")
    qrtscr = nc.dram_tensor("qrtscr", [NQ, CO], f32, kind="Internal")

    # ---- DMA inputs ----
    key_sb = [inp.tile([128, NK], bf16, tag=f"key{t}", name=f"key{t}") for t in range(2)]
    qry_sb = [inp.tile([128, NQ], bf16, tag=f"qry{t}", name=f"qry{t}") for t in range(2)]
    val_sb = [inp.tile([128, NK], bf16, tag=f"val{t}", name=f"val{t}") for t in range(2)]
    wkt_sb = [inp.tile([128, CO], bf16, tag=f"wkt{t}", name=f"wkt{t}") for t in range(2)]
    wqt_sb = [inp.tile([128, CO], bf16, tag=f"wqt{t}", name=f"wqt{t}") for t in range(2)]
    bqk_sb = consts.tile([CO, 1], f32, tag="bqk")
    wf32p_sb = consts.tile([128, NS * NG], bf16, tag="wf32p")
    bf2_sb = consts.tile([128, 1], f32, tag="bf2")
    ident = consts.tile([128, 128], f32, tag="ident")
    ident_bf = consts.tile([128, 128], bf16, tag="ident_bf")
    # critical-path inputs on the SP queue first; bulky value + consts on
    # the ACT/gpsimd queues so the k/q pipelines start ASAP
    for t in range(2):
        sl = slice(t * 128, (t + 1) * 128)
        nc.sync.dma_start(out=key_sb[t][:], in_=keyb.ap()[sl, :])
        nc.sync.dma_start(out=wkt_sb[t][:], in_=wkt.ap()[sl, :])
        nc.sync.dma_start(out=qry_sb[t][:], in_=qryb.ap()[sl, :])
        nc.sync.dma_start(out=wqt_sb[t][:], in_=wqt.ap()[sl, :])
    nc.scalar.dma_start(out=bqk_sb[:], in_=bqk.ap())
    nc.scalar.dma_start(out=wf32p_sb[:], in_=wf32p.ap())
    nc.scalar.dma_start(out=bf2_sb[:], in_=bf2.ap())
    for t in range(2):
        sl = slice(t * 128, (t + 1) * 128)
        nc.gpsimd.dma_start(out=val_sb[t][:], in_=valb.ap()[sl, :])
    make_identity(nc, ident[:])
    make_identity(nc, ident_bf[:])

    ksbig = work.tile([128, NS * NK], bf16, tag="ksbig")
    qb_big = work.tile([128, NGRP * NS], f32, tag="qb_big")
    k2n_sb = work.tile([CO, NK], bf16, tag="k2n")
    qn_sb = work.tile([CO, NQ], f32, tag="qn")
    vt_sb = [
        work.tile([KT_SIZES[kt], 2 * 128], bf16, tag=f"vt{kt}", name=f"vt{kt}")
        for kt in range(5)
    ]
    attn_sb = [
        work.tile([KT_SIZES[kt], NQ], bf16, tag=f"attn{kt}", name=f"attn{kt}")
        for kt in range(5)
    ]

    with tc.tile_pool(name="ppro", bufs=2, space="PSUM") as ppro:
        # ---- k_ = WkT^T @ key -> (64, 576) bf16, then to DRAM scratch ----
        for half in range(2):
            pk2 = ppro.tile([CO, NQ], f32, tag="ppro")
            csl = slice(half * NQ, (half + 1) * NQ)
            for ct in range(2):
                nc.tensor.matmul(
                    out=pk2[:],
                    lhsT=wkt_sb[ct][:],
                    rhs=key_sb[ct][:, csl],
                    start=(ct == 0),
                    stop=(ct == 1),
                )
            nc.vector.tensor_copy(out=k2n_sb[:, csl], in_=pk2[:])
            nc.gpsimd.dma_start(
                out=k2scr.ap()[:, csl], in_=k2n_sb[:, csl]
            )

        # ---- q_ + bq + bk -> (64, 288) f32 ----
        pqn = ppro.tile([CO, NQ], f32, tag="ppro")
        for ct in range(2):
            nc.tensor.matmul(
                out=pqn[:],
                lhsT=wqt_sb[ct][:],
                rhs=qry_sb[ct][:],
                start=(ct == 0),
                stop=(ct == 1),
            )
        nc.vector.tensor_scalar_add(out=qn_sb[:], in0=pqn[:], scalar1=bqk_sb[:])

        # ---- replicate k rows: ksbig[32*rho + u, s*576 + k] = k_[16*rho + s, k]
        # (channel of (band rho, step s) is c = 16*rho + s, block-contiguous,
        # so each band's source is a plain row range of k2scr; the gpsimd
        # queue issues these so the SP queue stays free for qb scatter) ----
        for kh in range(2):  # k-half outer: first half usable after first k2 copy
            for rho in range(RP):
                dst = ksbig[NG * rho : NG * (rho + 1), :].rearrange(
                    "p (s k) -> p s k", k=NK
                )[:, :, kh * NQ : (kh + 1) * NQ]
                srcap = (
                    k2scr.ap()[NS * rho : NS * (rho + 1), kh * NQ : (kh + 1) * NQ]
                    .rearrange("s k -> () s k")
                    .broadcast_to((NG, NS, NQ))
                )
                (nc.scalar if rho % 2 else nc.sync).dma_start(out=dst, in_=srcap)

        # ---- qRT = q_^T (q-part, c-free) via PE transpose ----
        qrt = work.tile([128, 3 * CO], f32, tag="qrt")
        for t in range(3):
            qsz = 128 if t < 2 else 32
            pqt = ppro.tile([128, CO], f32, tag="ppro")
            nc.tensor.transpose(
                out=pqt[:qsz, :],
                in_=qn_sb[:, t * 128 : t * 128 + qsz],
                identity=ident[:CO, :CO],
            )
            nc.vector.tensor_copy(
                out=qrt[:qsz, t * CO : (t + 1) * CO], in_=pqt[:qsz, :]
            )

        # ---- qb_big[32*rho + u, 16*G + s] = q_[16*rho + s, 32*G + u]
        # qrt -> DRAM (q, c) scratch (3 writes), then one clean strided read
        # per band: src[u, G, s] = qrtscr[32G + u, 16rho + s] ----
        for t in range(3):
            qsz = 128 if t < 2 else 32
            (nc.scalar if t % 2 else nc.sync).dma_start(
                out=qrtscr.ap()[t * 128 : t * 128 + qsz, :],
                in_=qrt[:qsz, t * CO : (t + 1) * CO],
            )
        for rho in range(RP):
            srcap = qrtscr.ap().rearrange("(g u) c -> u g c", u=NG)[
                :, :, 16 * rho : 16 * (rho + 1)
            ]
            (nc.scalar if rho % 2 else nc.sync).dma_start(
                out=qb_big[NG * rho : NG * (rho + 1), :].rearrange(
                    "p (g s) -> p g s", s=NS
                ),
                in_=srcap,
            )

        # ---- value transpose -> vT (k, cv) bf16 tiles ----
        for kt in range(5):
            pvt = ppro.tile([KT_SIZES[kt], 2 * 128], bf16, tag="pprobf")
            ks = slice(kt * 128, kt * 128 + KT_SIZES[kt])
            for ct in range(2):
                nc.tensor.transpose(
                    out=pvt[:, ct * 128 : (ct + 1) * 128],
                    in_=val_sb[ct][:, ks],
                    identity=ident_bf[:],
                )
            nc.vector.tensor_copy(out=vt_sb[kt][:], in_=pvt[:])

    # ---- main loop over 9 q-groups ----
    with (
        tc.tile_pool(name="pre", bufs=4) as prep,
        tc.tile_pool(name="aq", bufs=2) as aqp,
        tc.tile_pool(name="scp", bufs=2, space="PSUM") as scp,
        tc.tile_pool(name="patt", bufs=2, space="PSUM") as pattp,
    ):
        SQ = 8  # s-steps per tanh chunk (PE consumes batch n while ACT runs n+1)
        for G in range(NGRP):
            scg = scp.tile([NG, NK], f32, tag="scg")
            for sq in range(NS // SQ):
                pre = prep.tile([128, SQ * NK], bf16, tag="pre")
                for i in range(SQ):
                    s = sq * SQ + i
                    nc.vector.tensor_scalar_add(
                        out=pre[:, i * NK : (i + 1) * NK],
                        in0=ksbig[:, s * NK : (s + 1) * NK],
                        scalar1=qb_big[:, NS * G + s : NS * G + s + 1],
                    )
                nc.scalar.activation(pre[:], pre[:], AF.Tanh)
                for i in range(SQ):
                    s = sq * SQ + i
                    for c0, c1 in ((0, 512), (512, NK)):
                        nc.tensor.matmul(
                            out=scg[:, c0:c1],
                            lhsT=wf32p_sb[:, NG * s : NG * (s + 1)],
                            rhs=pre[:, i * NK + c0 : i * NK + c1],
                            start=(s == 0),
                            stop=(s == NS - 1),
                        )
            attn_q = aqp.tile([NG, NK], bf16, tag="attn_q")
            nc.scalar.activation(
                attn_q[:], scg[:], AF.Sigmoid, bias=bf2_sb[:NG, :]
            )
            for kt in range(5):
                ks = slice(kt * 128, kt * 128 + KT_SIZES[kt])
                patt = pattp.tile([KT_SIZES[kt], NG], bf16, tag="patt")
                nc.tensor.transpose(
                    out=patt[:], in_=attn_q[:, ks], identity=ident_bf[:NG, :NG]
                )
                nc.vector.tensor_copy(
                    out=attn_sb[kt][:, NG * G : NG * (G + 1)], in_=patt[:]
                )

    # ---- out = value @ attn : (256, 288) ----
    with tc.tile_pool(name="pout", bufs=2, space="PSUM") as pout:
        for ct in range(2):
            po = pout.tile([128, NQ], f32, tag="pout")
            for kt in range(5):
                nc.tensor.matmul(
                    out=po[:],
                    lhsT=vt_sb[kt][:, ct * 128 : (ct + 1) * 128],
                    rhs=attn_sb[kt][:],
                    start=(kt == 0),
                    stop=(kt == 4),
                )
            o_sb = work.tile([128, NQ], f32, tag=f"osb{ct}", name=f"osb{ct}")
            nc.vector.tensor_copy(out=o_sb[:], in_=po[:])
            nc.sync.dma_start(out=out.ap()[ct * 128 : (ct + 1) * 128, :], in_=o_sb[:])


def _build_pair(nc, mybir, tc, consts, inp, work, mode, chunk):
    """Older 2x64 pair layouts: mode 'bias' (ACT bias adds) or 'dve'."""
    from concourse.masks import make_identity

    f32 = mybir.dt.float32
    bf16 = mybir.dt.bfloat16
    AF = mybir.ActivationFunctionType

    keyb = nc.dram_tensor("keyb", [C, NK], f32, kind="ExternalInput")
    qryb = nc.dram_tensor("qryb", [C, NQ], f32, kind="ExternalInput")
    valb = nc.dram_tensor("valb", [C, NK], f32, kind="ExternalInput")
    wkt2 = nc.dram_tensor("wkt2", [C, 128], f32, kind="ExternalInput")
    wqt = nc.dram_tensor("wqt", [C, CO], f32, kind="ExternalInput")
    bqk2 = nc.dram_tensor("bqk2", [128, 1], f32, kind="ExternalInput")
    wf2 = nc.dram_tensor("wf2", [128, 2], bf16, kind="ExternalInput")
    bf2 = nc.dram_tensor("bf2", [128, 1], f32, kind="ExternalInput")
    out = nc.dram_tensor("out", [C, NQ], f32, kind="ExternalOutput")

    key_sb = [inp.tile([128, NK], f32, tag=f"key{t}", name=f"key{t}") for t in range(2)]
    qry_sb = [inp.tile([128, NQ], f32, tag=f"qry{t}", name=f"qry{t}") for t in range(2)]
    val_sb = [inp.tile([128, NK], f32, tag=f"val{t}", name=f"val{t}") for t in range(2)]
    wkt2_sb = [consts.tile([128, 128], f32, tag=f"wkt{t}", name=f"wkt{t}") for t in range(2)]
    wqt_sb = [consts.tile([128, CO], f32, tag=f"wqt{t}", name=f"wqt{t}") for t in range(2)]
    bqk2_sb = consts.tile([128, 1], f32, tag="bqk2")
    wf2_sb = consts.tile([128, 2], bf16, tag="wf2")
    bf2_sb = consts.tile([128, 1], f32, tag="bf2")
    ident = consts.tile([128, 128], f32, tag="ident")
    for t in range(2):
        sl = slice(t * 128, (t + 1) * 128)
        nc.sync.dma_start(out=key_sb[t][:], in_=keyb.ap()[sl, :])
        nc.sync.dma_start(out=qry_sb[t][:], in_=qryb.ap()[sl, :])
        nc.sync.dma_start(out=val_sb[t][:], in_=valb.ap()[sl, :])
        nc.sync.dma_start(out=wkt2_sb[t][:], in_=wkt2.ap()[sl, :])
        nc.sync.dma_start(out=wqt_sb[t][:], in_=wqt.ap()[sl, :])
    nc.sync.dma_start(out=bqk2_sb[:], in_=bqk2.ap())
    nc.sync.dma_start(out=wf2_sb[:], in_=wf2.ap())
    nc.sync.dma_start(out=bf2_sb[:], in_=bf2.ap())
    make_identity(nc, ident[:])

    with tc.tile_pool(name="ppro", bufs=1, space="PSUM") as ppro:
        k2_sb = work.tile([128, NK], bf16, tag="k2")
        for half in range(2):
            pk2 = ppro.tile([128, NQ], f32, tag="ppro")
            csl = slice(half * NQ, (half + 1) * NQ)
            for ct in range(2):
                nc.tensor.matmul(
                    out=pk2[:],
                    lhsT=wkt2_sb[ct][:],
                    rhs=key_sb[ct][:, csl],
                    start=(ct == 0),
                    stop=(ct == 1),
                )
            nc.vector.tensor_copy(out=k2_sb[:, csl], in_=pk2[:])

        pqb = ppro.tile([128, NPAIR], f32, tag="ppro")
        for par in range(2):
            for ct in range(2):
                nc.tensor.matmul(
                    out=pqb[par * CO : (par + 1) * CO, :],
                    lhsT=wqt_sb[ct][:],
                    rhs=qry_sb[ct][:, par : NQ : 2],
                    start=(ct == 0),
                    stop=(ct == 1),
                )
        qbias = work.tile([128, NPAIR], f32, tag="qbias")
        nc.vector.tensor_scalar_add(out=qbias[:], in0=pqb[:], scalar1=bqk2_sb[:])

        vt_sb = [
            work.tile([KT_SIZES[kt], 2 * 128], bf16, tag=f"vt{kt}", name=f"vt{kt}")
            for kt in range(5)
        ]
        for kt in range(5):
            pvt = ppro.tile([KT_SIZES[kt], 2 * 128], bf16, tag="pprobf")
            ks = slice(kt * 128, kt * 128 + KT_SIZES[kt])
            for ct in range(2):
                nc.tensor.transpose(
                    out=pvt[:, ct * 128 : (ct + 1) * 128],
                    in_=val_sb[ct][:, ks],
                    identity=ident_bf[:],
                )
            nc.vector.tensor_copy(out=vt_sb[kt][:], in_=pvt[:])

    attn_sb = [
        work.tile([KT_SIZES[kt], NQ], bf16, tag=f"attn{kt}", name=f"attn{kt}")
        for kt in range(5)
    ]

    with tc.tile_pool(name="psc", bufs=1, space="PSUM") as psc:
        psc_t = [
            psc.tile([KT_SIZES[kt], NQ], f32, tag=f"sc{kt}", name=f"sc{kt}")
            for kt in range(5)
        ]
        if mode == "bias":
            with tc.tile_pool(name="pre", bufs=3) as prep:
                for j in range(NPAIR):
                    pre2 = prep.tile([128, NK], bf16, tag="pre2")
                    nc.scalar.activation(
                        pre2[:], k2_sb[:], AF.Tanh, bias=qbias[:, j : j + 1]
                    )
                    for kt in range(5):
                        ks = slice(kt * 128, kt * 128 + KT_SIZES[kt])
                        nc.tensor.matmul(
                            out=psc_t[kt][:, 2 * j : 2 * j + 2],
                            lhsT=pre2[:, ks],
                            rhs=wf2_sb[:],
                            start=True,
                            stop=True,
                        )
        else:  # dve
            nchunk = (NPAIR + chunk - 1) // chunk
            with tc.tile_pool(name="pre", bufs=2) as prep:
                for cidx in range(nchunk):
                    j0 = cidx * chunk
                    j1 = min(j0 + chunk, NPAIR)
                    pre = prep.tile([128, chunk * NK], bf16, tag="pre")
                    for j in range(j0, j1):
                        sl = slice((j - j0) * NK, (j - j0 + 1) * NK)
                        nc.vector.tensor_scalar_add(
                            out=pre[:, sl], in0=k2_sb[:], scalar1=qbias[:, j : j + 1]
                        )
                    nc.scalar.activation(
                        pre[:, : (j1 - j0) * NK], pre[:, : (j1 - j0) * NK], AF.Tanh
                    )
                    for j in range(j0, j1):
                        for kt in range(5):
                            ks = slice(
                                (j - j0) * NK + kt * 128,
                                (j - j0) * NK + kt * 128 + KT_SIZES[kt],
                            )
                            nc.tensor.matmul(
                                out=psc_t[kt][:, 2 * j : 2 * j + 2],
                                lhsT=pre[:, ks],
                                rhs=wf2_sb[:],
                                start=True,
                                stop=True,
                            )
        for kt in range(5):
            nc.scalar.activation(
                attn_sb[kt][:],
                psc_t[kt][:],
                AF.Sigmoid,
                bias=bf2_sb[: KT_SIZES[kt], :],
            )

    with tc.tile_pool(name="pout", bufs=2, space="PSUM") as pout:
        for ct in range(2):
            po = pout.tile([128, NQ], f32, tag="pout")
            for kt in range(5):
                nc.tensor.matmul(
                    out=po[:],
                    lhsT=vt_sb[kt][:, ct * 128 : (ct + 1) * 128],
                    rhs=attn_sb[kt][:],
                    start=(kt == 0),
                    stop=(kt == 4),
                )
            o_sb = work.tile([128, NQ], f32, tag=f"osb{ct}", name=f"osb{ct}")
            nc.vector.tensor_copy(out=o_sb[:], in_=po[:])
            nc.sync.dma_start(out=out.ap()[ct * 128 : (ct + 1) * 128, :], in_=o_sb[:])


def _build(mode="rep", chunk=12):
    import concourse.bacc as bacc
    import concourse.mybir as mybir
    from concourse.tile import TileContext

    nc = bacc.Bacc("TRN2", target_bir_lowering=False, debug=False, num_devices=8)
    with TileContext(nc) as tc:
        with (
            tc.tile_pool(name="consts", bufs=1) as consts,
            tc.tile_pool(name="inp", bufs=1) as inp,
            tc.tile_pool(name="work", bufs=1) as work,
        ):
            if mode == "rep":
                _build_rep(nc, mybir, tc, consts, inp, work)
            else:
                _build_pair(nc, mybir, tc, consts, inp, work, mode, chunk)
    nc.finalize()
    return nc


def _prep_in_maps(mode, key, query, value, Wk, bk, Wq, bq, wf, bf):
    import ml_dtypes

    f32 = np.float32
    key = np.ascontiguousarray(key, f32).reshape(B, C, NK)
    query = np.ascontiguousarray(query, f32).reshape(B, C, HW, HW)
    value = np.ascontiguousarray(value, f32).reshape(B, C, NK)
    WqT = np.ascontiguousarray(np.asarray(Wq, f32).T)  # (256, 64)
    bf2 = np.full((128, 1), np.float32(bf), f32)
    wf = np.asarray(wf, f32)

    common = {"wqt": WqT, "bf2": bf2}
    if mode == "rep":
        import ml_dtypes as mld

        common["wqt"] = WqT.astype(mld.bfloat16)
        common["wkt"] = np.ascontiguousarray(np.asarray(Wk, f32).T).astype(mld.bfloat16)
        common["bqk"] = (np.asarray(bk, f32) + np.asarray(bq, f32)).reshape(CO, 1)
        wf32p = np.zeros((128, NS, NG), f32)
        for rho in range(RP):
            for s in range(NS):
                # channel of (band rho, step s) is 16*rho + s (block-contiguous)
                wf32p[NG * rho : NG * (rho + 1), s, :] = np.eye(NG, dtype=f32) * wf[
                    NS * rho + s
                ]
        common["wf32p"] = np.ascontiguousarray(
            wf32p.reshape(128, NS * NG).astype(ml_dtypes.bfloat16)
        )
    else:
        common["wkt2"] = np.ascontiguousarray(
            np.concatenate([np.asarray(Wk, f32).T] * 2, axis=1)
        )
        common["bqk2"] = np.ascontiguousarray(
            np.tile(np.asarray(bk, f32) + np.asarray(bq, f32), 2).reshape(128, 1)
        )
        wf2 = np.zeros((128, 2), f32)
        wf2[:CO, 0] = wf
        wf2[CO:, 1] = wf
        common["wf2"] = wf2.astype(ml_dtypes.bfloat16)

    if mode == "rep":
        import ml_dtypes as mld

        key = key.astype(mld.bfloat16)
        query = query.astype(mld.bfloat16)
        value = value.astype(mld.bfloat16)
    in_maps = []
    for i in range(8):
        b, h = i // 2, i % 2
        qs = np.ascontiguousarray(query[b, :, h * 12 : (h + 1) * 12, :]).reshape(C, NQ)
        m = {"keyb": np.ascontiguousarray(key[b]), "qryb": qs, "valb": np.ascontiguousarray(value[b])}
        m.update(common)
        in_maps.append(m)
    return in_maps


def run(mode="rep", chunk=12, trace=False, **inputs):
    from concourse.bass_utils import run_bass_kernel_spmd

    cache_key = (mode, chunk)
    if cache_key not in _cache:
        _cache[cache_key] = _build(mode, chunk)
    nc = _cache[cache_key]
    in_maps = _prep_in_maps(mode, **inputs)
    res = run_bass_kernel_spmd(nc, in_maps, core_ids=list(range(8)), trace=trace)
    out = np.empty((B, C, HW, HW), np.float32)
    for i in range(8):
        b, h = i // 2, i % 2
        out[b, :, h * 12 : (h + 1) * 12, :] = res.results[i]["out"].reshape(C, 12, HW)
    return out, res


def kernel(**inputs):
    out, _ = run(mode="rep", **inputs)
    return out
